# revision 35
# baseline (speedup 1.0000x reference)
"""Multi-head self-attention Trainium2 kernel (Bass/Tile), batch-sharded SPMD.

Problem: seq [2048, 8, 512] fp32, fused QKV (W_qkv [1536,512], b_qkv [1536]),
H=8 heads of HD=64, full softmax attention, out proj (W_out [512,512], b_out).

Sharding: batch (bs=8) across 8 NeuronCores, one batch element per core, no
collectives. The host pre-transposes per-core x -> xT [e, n] and the weights
(and casts them to bf16), scatters, and gathers y -> [n, bs, e].

Per-core dataflow (n=2048, E=512, all matmuls bf16 with fp32 PSUM):
  qkT [f, n] <- WqkvT.T @ xT   (f in [0,1024): q|k features; each 128-row
                tile holds a head PAIR: rows 0:64 head 2p, 64:128 head 2p+1)
  v   [n, f] <- xT.T @ WvT     (no bias matmul: since sum(softmax)=1, the
                v-bias is folded into the out-proj bias bo2 = b_out+bv@WoutT)
  attention, per head pair p, per q-chunk (512 cols), 3-kb cycles:
    scoresT[k,q]: row-paired matmuls into per-kb streams sA/sB/sC
    exp: balanced across ScalarE (exact ACTIVATE) and DVE (custom EXP16_ANT,
         exp(s/8) ~ poly^16) at 9:7 per qc - the v1 split (ACT ~2.2us/cycle
         vs the 1.94us PE cycle) made ACT the wall and stalled each cycle
    av/denominator (deferred one cycle): col-paired matmuls po += v.T @ e,
        pd += ones.T @ e; epilogue rc=1/pd on DVE, outT = po*rc on DVE
  y [n, f] = outT.T @ WoutT + b: bias enters as a K=1 ones_row x bo2 matmul
    into PSUM, so the epilogue is a pure PSUM->SBUF copy split ScalarE/DVE.

Changes vs the 302.7us prior version (trace-driven; measured 297.0us,
rel err 7.0e-3):
  - 2-kb cycles (was 3): each cycle carries exactly one ScalarE exp and one
    DVE exp (1114/1224ns, both under the ~1.3us PE cycle), killing the
    per-cycle integer imbalance (2 ACT exps = 2.23us/cycle vs a 1.94us
    3-kb PE cycle) that stalled the PE ~700-900ns every cycle
  - cycle 0 of each qc is [ACT,ACT]: the DVE gap absorbs the previous qc's
    recip+mul so the po/pd WAR at the qc boundary stops stalling the PE
  - score streams rotate by GLOBAL kb index (gkb%3) so the first stream a
    new qc reuses was exp'd 3 kb earlier, not at the previous qc's end
  - exp split 9:7 ScalarE:DVE per qc (engine-balanced incl. normalize)
  - startup: j=0 slivers of W-fb0/x-ncol0 land first; W ordered fb0 -> v
    slice -> the rest to match qk(0) -> v -> qk(4..) consumption; PE warm-up
    matmuls during the DMA dead time so the HAM clock gate opens early
  - qk(7) n0/n1 ride the attention deferred-work queue (their PSUM borrows
    the o/d banks) to fill the first qc's exp pipeline-fill bubble
  - persistent tiles consolidated (qkT/v/outT single tiles): smaller exit
    barrier at the tail

Where the remaining time goes (per the NTFF profile): PE streaming floor
~218us + ~30us exposed LDWEIGHTS at paired-matmul quadrant transitions
(tile_position'd loads cannot use the background weight buffer) + ~47us
projection phase + ~6us fixed runtime setup + ~6us tail teardown.  The
attention phase runs cycle-lockstep (3 single-buffered score streams), so
per-cycle time = max(PE, exp engine) every cycle; PE and the two exp
engines are co-bound within ~10%.  Rejected with numbers: fp8/DoubleRow
anywhere in the value path (~2.5% rel err - quantization of a random
weighted sum passes through 1:1, gate is 2e-2), GpSimd softmax-denominator
offload (partition_all_reduce 13.2us per [128,2048]; tensor_add 2.1us per
[128,1024] - capacity-dead), and e-pair pre-summing for pd (PE savings in
light cycles don't transfer across the lockstep; measured neutral).
"""

import numpy as np

import concourse.bass as bass
import concourse.mybir as mybir
import concourse.tile as tile
from concourse import bacc
from concourse import dve_ops
from concourse.dve_spec import Spec, Src0, C0, C1, C2, sq
from concourse.dve_uop import DveOpSpec
from concourse.dve_ops import DveOp
from concourse.dve_spec import lower as dve_lower

F32 = mybir.dt.float32
BF16 = mybir.dt.bfloat16

N_SEQ, BS, E, H, HD = 2048, 8, 512, 8, 64
N_CORES = 8

# exp(0.125*s) ~ ((EC2*s + EC1)*s + EC0)^16, minimax-fitted on s in [-40, 40]
# (observed raw-score range is [-36.3, 37.2]); max rel err 2.05e-2 which lands
# at ~6.7e-3 end-to-end with 7/16 of k-blocks routed to the DVE.
EC2, EC1, EC0 = 3.03313468e-05, 7.90702397e-03, 1.00029378e+00


def _register_exp16():
    """Register the custom DVE op (documented extension point in dve_ops)."""
    if "EXP16_ANT" in dve_ops._SUB_OPCODE_FOR_NAME:
        return next(o for o in dve_ops.OPS if o.name == "EXP16_ANT")
    body = sq(sq(sq(sq((Src0 * C0 + C1) * Src0 + C2))))

    def ref(in0, in1, s0, s1, imm2):
        p = (in0.astype(np.float32) * s0 + s1) * in0 + imm2
        for _ in range(4):
            p = p * p
        return p

    spec = Spec(body=body, reference=ref)
    shas = {}
    for ver in ("v3", "v4"):
        uops = dve_lower(spec, ver=ver)
        shas[ver] = DveOpSpec(name="EXP16_ANT", opcode=0, uops=uops, rd1_en=False).sha(ver)
    op = DveOp("EXP16_ANT", spec, subdim=False, uops_sha=shas)
    dve_ops.OPS.append(op)
    dve_ops.CUSTOM_DVE_SPECS[op.name] = spec
    dve_ops._SUB_OPCODE_FOR_NAME[op.name] = (
        dve_ops._CUSTOM_DVE_ROW_BASE + len(dve_ops.OPS) - 1
    )
    return op


EXP16 = _register_exp16()


def _emit(tc, nc, xT_d, w_qkvT, b_qkv, w_outT, b_out, y, n):
    NB = n // 128   # token blocks
    QC = n // 512   # q chunks
    KB = n // 128   # k blocks
    EC = E // 128   # e chunks

    persist_cm = tc.tile_pool(name="persist", bufs=1)
    persist = persist_cm.__enter__()

    ones_col = persist.tile([128, 64], BF16, tag="ones_col", name="ones_col")
    nc.vector.memset(ones_col, 1.0)
    ones_row = persist.tile([1, 128], BF16, tag="ones_row", name="ones_row")
    nc.vector.memset(ones_row, 1.0)

    # biases: b_qkv[0:1024] per-partition [128, fb]; v-bias folded into the
    # output-projection bias (sum(softmax)=1): bo2 = b_out + bv @ WoutT
    bqk = persist.tile([128, 8], F32, tag="bqk", name="bqk")
    nc.gpsimd.dma_start(out=bqk, in_=b_qkv[0:1024].rearrange("(a b) -> b a", b=128))
    bv_col = persist.tile([128, 4], F32, tag="bv_col", name="bv_col")
    nc.gpsimd.dma_start(
        out=bv_col, in_=b_qkv[1024:1536].rearrange("(a b) -> b a", b=128)
    )
    bvb = persist.tile([128, 4], BF16, tag="bvb", name="bvb")
    nc.vector.tensor_copy(bvb, bv_col)
    bo_f = persist.tile([1, 512], F32, tag="bo_f", name="bo_f")
    nc.gpsimd.dma_start(out=bo_f, in_=b_out.unsqueeze(0))
    bo2 = persist.tile([1, 512], BF16, tag="bo2", name="bo2")
    bo2b = persist.tile([128, 512], F32, tag="bo2b", name="bo2b")
    wu_sb = persist.tile([128, 512], BF16, tag="wu_sb", name="wu_sb")
    nc.vector.memset(wu_sb, 0.0)

    # persistent bf16 operands (single tiles; fewer tags = smaller exit
    # barrier at the kernel tail)
    xT = persist.tile([128, EC, n], BF16, tag="xT", name="xT")
    wqkvT = persist.tile([128, EC, 1536], BF16, tag="wqkvT", name="wqkvT")
    woutT = persist.tile([128, EC, 512], BF16, tag="woutT", name="woutT")
    qkT = persist.tile([128, 8, n], BF16, tag="qkT", name="qkT")
    v_sb = persist.tile([128, NB, 512], BF16, tag="v_sb", name="v_sb")
    outT = persist.tile([128, 4, n], BF16, tag="outT", name="outT")

    # ---------------- phase 0: load (bf16, pre-transposed on host) + QKV ----
    with (
        tc.tile_pool(name="pqkv", bufs=4, space="PSUM") as pqkv_pool,
    ):
        # Critical-path loads first: the very first matmul only needs the
        # j=0 slivers of W-fb0 and x-ncol0, so those are their own tiny DMAs.
        # Then the j-rest of each, the full-width W lines (split across the
        # sync and gpsimd queues so they land before the v matmuls), the
        # remaining x chunks, and W_out last.
        wq_r = w_qkvT.rearrange("(j p) c -> p j c", p=128)
        x_r = xT_d.rearrange("(j p) c -> p j c", p=128)
        # PE warm-up during the DMA dead time: the HAM clock gate opens after
        # ~3.4us of sustained PE activity, so spin a few matmuls on memset
        # data and the first real matmuls run at 2.4GHz instead of 1.2.
        # Enough warm-up matmuls to stay busy until the first x/W data lands
        # (~11-13us): a PE idle gap after the warm-ups would reset the HAM
        # activity window and the first ~20 real matmuls would run at 1.2GHz.
        for wi in range(10):
            pw = pqkv_pool.tile([128, 512], F32, tag="qk" if wi % 2 else "v",
                                name="pw")
            nc.tensor.matmul(pw[0:64, :], lhsT=ones_col, rhs=wu_sb,
                             start=True, stop=True)
        # DMA order matches consumption: qk(0) [W fb0 + x ncol0], v [W
        # 1024:1536], qk(4..) [W 128:1024].  Whole multi-j chunks: many small
        # descriptors spread across all 8 DMA engines, so a [128,4,512] chunk
        # lands EARLIER than a "critical-path" [128,1,512] sliver would.
        nc.sync.dma_start(out=wqkvT[:, :, 0:128], in_=wq_r[:, :, 0:128])
        nc.scalar.dma_start(out=xT[:, :, 0:512], in_=x_r[:, :, 0:512])
        for j in range(EC):
            nc.sync.dma_start(
                out=wqkvT[:, j, 1024:1536], in_=w_qkvT[j * 128:(j + 1) * 128, 1024:1536]
            )
        for j in range(EC):
            nc.sync.dma_start(
                out=wqkvT[:, j, 128:1024], in_=w_qkvT[j * 128:(j + 1) * 128, 128:1024]
            )
        for ncol in range(1, QC):
            nc.scalar.dma_start(
                out=xT[:, :, ncol * 512:(ncol + 1) * 512],
                in_=x_r[:, :, ncol * 512:(ncol + 1) * 512],
            )
        for j in range(EC):
            nc.gpsimd.dma_start(
                out=woutT[:, j, :], in_=w_outT[j * 128:(j + 1) * 128, :]
            )

        def emit_qk_ncol(fb, ncol, pq=None):
            if pq is None:
                pq = pqkv_pool.tile([128, 512], F32, tag="qk", name="pq")
            for j in range(EC):
                nc.tensor.matmul(
                    pq,
                    lhsT=wqkvT[:, j, fb * 128:(fb + 1) * 128],
                    rhs=xT[:, j, ncol * 512:(ncol + 1) * 512],
                    start=(j == 0),
                    stop=(j == EC - 1),
                )
            nc.vector.tensor_scalar_add(
                qkT[:, fb, ncol * 512:(ncol + 1) * 512], pq, bqk[:, fb:fb + 1]
            )

        def emit_qk(fb):
            for ncol in range(QC):
                emit_qk_ncol(fb, ncol)

        def emit_v(nb):
            pv = pqkv_pool.tile([128, 512], F32, tag="v", name="pv")
            for j in range(EC):
                nc.tensor.matmul(
                    pv,
                    lhsT=xT[:, j, nb * 128:(nb + 1) * 128],
                    rhs=wqkvT[:, j, 1024:1536],
                    start=(j == 0),
                    stop=(j == EC - 1),
                )
            nc.vector.tensor_copy(v_sb[:, nb, :], pv)

        emit_qk(0)
        for nb in range(NB):
            emit_v(nb)
        emit_qk(4)
        for fb in (1, 5, 2, 6, 3):
            emit_qk(fb)
        for ncol in (2, 3):
            emit_qk_ncol(7, ncol)

        # bo2 = b_out + bv @ WoutT (one-time; replaces the separate v-bias).
        # Emitted last so its woutT dependency never blocks the qk/v stream.
        pb = pqkv_pool.tile([128, 512], F32, tag="qk", name="pb")
        for j in range(EC):
            nc.tensor.matmul(
                pb[0:1, :], lhsT=bvb[:, j:j + 1], rhs=woutT[:, j, :],
                start=(j == 0), stop=(j == EC - 1),
            )
        nc.vector.tensor_add(bo2, bo_f, pb[0:1, :])
        # broadcast bo2 over 128 partitions once: the finals then add it on
        # the DVE copy instead of spending a PE slot per block on a rank-1
        # ones-lhsT matmul
        pbb = pqkv_pool.tile([128, 512], F32, tag="qk", name="pbb")
        nc.tensor.matmul(pbb, lhsT=ones_row, rhs=bo2, start=True, stop=True)
        nc.vector.tensor_copy(bo2b, pbb)

    # ---------------- phase 1: attention ----------------
    # 2-kb cycles: each cycle computes two kb's score pairs and (one cycle
    # deferred) their av/pd matmuls - 6 pair-slots ~ 1.29us of PE work - and
    # issues exactly one exp per engine (ScalarE 1.11us, DVE 1.22us), so
    # neither engine ever backlogs.  Cycle 0 of each qc is [ACT,ACT]: the
    # DVE gap there absorbs the previous qc's recip+mul, so the po/pd WAR
    # at the qc boundary resolves before av(kb0) needs the banks.  Score
    # streams rotate by GLOBAL kb (gkb%3): the stream a new qc's kb0 reuses
    # was exp'd 3 kb earlier, not at the previous qc's end.
    STAG = ("sA", "sB", "sC")
    cycles = [tuple(range(s, s + 2)) for s in range(0, KB, 2)]
    with (
        tc.tile_pool(name="ps", bufs=1, space="PSUM") as s_pool,
        tc.tile_pool(name="po", bufs=1, space="PSUM") as o_pool,
        tc.tile_pool(name="se", bufs=3) as e_pool,
        tc.tile_pool(name="sr", bufs=2) as r_pool,
        tc.tile_pool(name="sy", bufs=4) as y_pool,
    ):
        def emit_final(nb, ftag):
            pf = o_pool.tile([128, 512], F32, tag=ftag, name="pf")
            for pp in range(4):
                nc.tensor.matmul(
                    pf, lhsT=outT[:, pp, nb * 128:(nb + 1) * 128],
                    rhs=woutT[:, pp, :], start=(pp == 0), stop=(pp == 3),
                )
            ys = y_pool.tile([128, 512], F32, tag="y", name="ys")
            nc.vector.tensor_add(ys, pf, bo2b)
            nc.sync.dma_start(out=y[nb * 128:(nb + 1) * 128, :], in_=ys)

        # deferred-work queue: (weight in PE pair-slots, closure).  Each
        # cycle flushes ~one cycle's worth so avs trail their exps by one
        # cycle and the finals spread instead of bunching.
        work = []

        def flush(budget=5):
            spent = 0
            while work and spent < budget:
                wt, w = work.pop(0)
                w()
                spent += wt

        # all of qk(7) fills the first qc's pipeline-fill bubble (the PE has
        # no deferred avs in its first cycles, and an idle gap there lets the
        # HAM clock gate re-throttle); its PSUM rides the o/d banks, whose
        # first real write av(kb0) is emitted after these flush
        def qk7_fill(ncol, ftag):
            pq = o_pool.tile([128, 512], F32, tag=ftag, name="pq7")
            emit_qk_ncol(7, ncol, pq=pq)

        for ncol in range(2):
            work.append((4, lambda ncol=ncol, t=("o" if ncol % 2 == 0 else "d"):
                         qk7_fill(ncol, t)))

        for p in range(4):
            for qc in range(QC):
                qs = slice(qc * 512, (qc + 1) * 512)
                po = o_pool.tile([128, 512], F32, tag="o", name="po")
                pd = o_pool.tile([128, 512], F32, tag="d", name="pd")

                def av(e, kb, po=po, pd=pd, p=p):
                    first, last = (kb == 0), (kb == KB - 1)
                    eA = e[:, 0, :]
                    eB = e[:, 1, :]

                    def po_mm():
                        nc.tensor.matmul(
                            po[0:64, :], lhsT=v_sb[:, kb, p * 128:p * 128 + 64],
                            rhs=eA, start=first, stop=last, skip_group_check=True,
                        )
                        nc.tensor.matmul(
                            po[64:128, :],
                            lhsT=v_sb[:, kb, p * 128 + 64:(p + 1) * 128],
                            rhs=eB, start=first, stop=last, skip_group_check=True,
                        )

                    def pd_mm():
                        nc.tensor.matmul(
                            pd[0:64, :], lhsT=ones_col, rhs=eA,
                            start=first, stop=last, skip_group_check=True,
                        )
                        nc.tensor.matmul(
                            pd[64:128, :], lhsT=ones_col, rhs=eB,
                            start=first, stop=last, skip_group_check=True,
                        )

                    # on the last k-block, finish pd first so the reciprocal
                    # in the epilogue starts two slots earlier
                    if last:
                        pd_mm(); po_mm()
                    else:
                        po_mm(); pd_mm()

                def normalize(po=po, pd=pd, p=p, qs=qs):
                    rc = r_pool.tile([128, 512], F32, tag="rc", name="rc")
                    nc.vector.reciprocal_approx_fast(rc, pd)
                    nc.vector.tensor_mul(outT[:, p, qs], po, rc)

                for ci, cyc in enumerate(cycles):
                    new_avs = []
                    for i, kb in enumerate(cyc):
                        gkb = ((p * QC + qc) * KB + kb)
                        st = STAG[gkb % 3]
                        ks = slice(kb * 128, (kb + 1) * 128)
                        S = s_pool.tile([128, 2, 512], F32, tag=st, name="S")
                        nc.tensor.matmul(
                            S[:, 0, :], lhsT=qkT[0:64, 4 + p, ks],
                            rhs=qkT[0:64, p, qs], start=True, stop=True,
                        )
                        nc.tensor.matmul(
                            S[:, 1, :], lhsT=qkT[64:128, 4 + p, ks],
                            rhs=qkT[64:128, p, qs], start=True, stop=True,
                        )
                        e = e_pool.tile([128, 2, 512], BF16, tag="e" + st, name="e")
                        if ci == 0:
                            on_dve = False          # [A, A] boundary cycle
                        elif ci % 2 == 1:
                            on_dve = (i == 0)       # [D, A]
                        else:
                            on_dve = (i == 1)       # [A, D]
                        if on_dve:
                            nc.vector._custom_dve(
                                EXP16, out=e, in0=S, s0=EC2, s1=EC1, imm2=EC0
                            )
                        else:
                            nc.scalar.activation(
                                e, S, mybir.ActivationFunctionType.Exp, scale=0.125,
                            )
                        new_avs.append(
                            (2, lambda e=e, kb=kb, av=av: av(e, kb))
                        )
                    flush()
                    work.extend(new_avs)
                # normalization and (on the last pair) the finished output
                # rows join the deferred queue so the next qc's scores/exps
                # stay ahead of them
                work.append((0, normalize))
                if p == 3:
                    for i, nb in enumerate(range(qc * 4, qc * 4 + 4)):
                        work.append(
                            (4, lambda nb=nb, t=("o" if i % 2 == 0 else "d"),
                                emit_final=emit_final: emit_final(nb, t))
                        )
        while work:
            flush()
    persist_cm.__exit__(None, None, None)


def build(n=N_SEQ):
    nc = bacc.Bacc("TRN2", target_bir_lowering=False, debug=False)
    xT_d = nc.dram_tensor("xT", [E, n], BF16, kind="ExternalInput").ap()
    w_qkvT = nc.dram_tensor("w_qkvT", [E, 3 * E], BF16, kind="ExternalInput").ap()
    b_qkv = nc.dram_tensor("b_qkv", [3 * E], F32, kind="ExternalInput").ap()
    w_outT = nc.dram_tensor("w_outT", [E, E], BF16, kind="ExternalInput").ap()
    b_out = nc.dram_tensor("b_out", [E], F32, kind="ExternalInput").ap()
    y = nc.dram_tensor("y", [n, E], F32, kind="ExternalOutput").ap()
    with tile.TileContext(nc) as tc:
        _emit(tc, nc, xT_d, w_qkvT, b_qkv, w_outT, b_out, y, n)
    nc.compile()
    return nc


_NC_CACHE = {}


def _get_nc(n):
    if n not in _NC_CACHE:
        _NC_CACHE[n] = build(n)
    return _NC_CACHE[n]


def _in_maps(seq, W_qkv, b_qkv, W_out, b_out):
    import ml_dtypes

    bf16 = ml_dtypes.bfloat16
    seq = np.asarray(seq, np.float32)
    wqT = np.ascontiguousarray(np.asarray(W_qkv, np.float32).T.astype(bf16))
    bq = np.ascontiguousarray(np.asarray(b_qkv, np.float32))
    woT = np.ascontiguousarray(np.asarray(W_out, np.float32).T.astype(bf16))
    bo = np.ascontiguousarray(np.asarray(b_out, np.float32))
    return [
        {
            "xT": np.ascontiguousarray(seq[:, b, :].T.astype(bf16)),  # [E, n]
            "w_qkvT": wqT,
            "b_qkv": bq,
            "w_outT": woT,
            "b_out": bo,
        }
        for b in range(seq.shape[1])
    ]


def run(seq, W_qkv, b_qkv, W_out, b_out, trace=False):
    """Returns (out [n, bs, e] fp32, BassKernelResults)."""
    from concourse.bass_utils import run_bass_kernel_spmd

    seq = np.asarray(seq, np.float32)
    n, bs, e = seq.shape
    nc = _get_nc(n)
    res = run_bass_kernel_spmd(
        nc,
        _in_maps(seq, W_qkv, b_qkv, W_out, b_out),
        core_ids=list(range(N_CORES)),
        trace=trace,
    )
    out = np.empty((n, bs, e), np.float32)
    for b in range(bs):
        out[:, b, :] = res.results[b]["y"]
    return out, res


def kernel(seq, W_qkv, b_qkv, W_out, b_out):
    out, _ = run(seq, W_qkv, b_qkv, W_out, b_out)
    return out


# revision 36
# speedup vs baseline: 1.0100x; 1.0100x over previous
"""Multi-head self-attention Trainium2 kernel (Bass/Tile), batch-sharded SPMD.

Problem: seq [2048, 8, 512] fp32, fused QKV (W_qkv [1536,512], b_qkv [1536]),
H=8 heads of HD=64, full softmax attention, out proj (W_out [512,512], b_out).

Sharding: batch (bs=8) across 8 NeuronCores, one batch element per core, no
collectives. The host pre-transposes per-core x -> xT [e, n] and the weights
(and casts them to bf16), scatters, and gathers y -> [n, bs, e].

Per-core dataflow (n=2048, E=512, all matmuls bf16 with fp32 PSUM):
  qkT [f, n] <- WqkvT.T @ xT   (f in [0,1024): q|k features; each 128-row
                tile holds a head PAIR: rows 0:64 head 2p, 64:128 head 2p+1)
  v   [n, f] <- xT.T @ WvT     (no bias matmul: since sum(softmax)=1, the
                v-bias is folded into the out-proj bias bo2 = b_out+bv@WoutT)
  attention, per head pair p, per q-chunk (512 cols), 3-kb cycles:
    scoresT[k,q]: row-paired matmuls into per-kb streams sA/sB/sC
    exp: balanced across ScalarE (exact ACTIVATE) and DVE (custom EXP16_ANT,
         exp(s/8) ~ poly^16) at 9:7 per qc - the v1 split (ACT ~2.2us/cycle
         vs the 1.94us PE cycle) made ACT the wall and stalled each cycle
    av/denominator (deferred one cycle): col-paired matmuls po += v.T @ e,
        pd += ones.T @ e; epilogue rc=1/pd on DVE, outT = po*rc on DVE
  y [n, f] = outT.T @ WoutT + b: bias enters as a K=1 ones_row x bo2 matmul
    into PSUM, so the epilogue is a pure PSUM->SBUF copy split ScalarE/DVE.

Changes vs the 302.7us prior version (trace-driven; measured 297.0us,
rel err 7.0e-3):
  - 2-kb cycles (was 3): each cycle carries exactly one ScalarE exp and one
    DVE exp (1114/1224ns, both under the ~1.3us PE cycle), killing the
    per-cycle integer imbalance (2 ACT exps = 2.23us/cycle vs a 1.94us
    3-kb PE cycle) that stalled the PE ~700-900ns every cycle
  - cycle 0 of each qc is [ACT,ACT]: the DVE gap absorbs the previous qc's
    recip+mul so the po/pd WAR at the qc boundary stops stalling the PE
  - score streams rotate by GLOBAL kb index (gkb%3) so the first stream a
    new qc reuses was exp'd 3 kb earlier, not at the previous qc's end
  - exp split 9:7 ScalarE:DVE per qc (engine-balanced incl. normalize)
  - startup: j=0 slivers of W-fb0/x-ncol0 land first; W ordered fb0 -> v
    slice -> the rest to match qk(0) -> v -> qk(4..) consumption; PE warm-up
    matmuls during the DMA dead time so the HAM clock gate opens early
  - qk(7) n0/n1 ride the attention deferred-work queue (their PSUM borrows
    the o/d banks) to fill the first qc's exp pipeline-fill bubble
  - persistent tiles consolidated (qkT/v/outT single tiles): smaller exit
    barrier at the tail

Where the remaining time goes (per the NTFF profile): PE streaming floor
~218us + ~30us exposed LDWEIGHTS at paired-matmul quadrant transitions
(tile_position'd loads cannot use the background weight buffer) + ~47us
projection phase + ~6us fixed runtime setup + ~6us tail teardown.  The
attention phase runs cycle-lockstep (3 single-buffered score streams), so
per-cycle time = max(PE, exp engine) every cycle; PE and the two exp
engines are co-bound within ~10%.  Rejected with numbers: fp8/DoubleRow
anywhere in the value path (~2.5% rel err - quantization of a random
weighted sum passes through 1:1, gate is 2e-2), GpSimd softmax-denominator
offload (partition_all_reduce 13.2us per [128,2048]; tensor_add 2.1us per
[128,1024] - capacity-dead), and e-pair pre-summing for pd (PE savings in
light cycles don't transfer across the lockstep; measured neutral).
"""

import numpy as np

import concourse.bass as bass
import concourse.mybir as mybir
import concourse.tile as tile
from concourse import bacc
from concourse import dve_ops
from concourse.dve_spec import Spec, Src0, C0, C1, C2, sq
from concourse.dve_uop import DveOpSpec
from concourse.dve_ops import DveOp
from concourse.dve_spec import lower as dve_lower

F32 = mybir.dt.float32
BF16 = mybir.dt.bfloat16

N_SEQ, BS, E, H, HD = 2048, 8, 512, 8, 64
N_CORES = 8

# exp(0.125*s) ~ ((EC2*s + EC1)*s + EC0)^16, minimax-fitted on s in [-40, 40]
# (observed raw-score range is [-36.3, 37.2]); max rel err 2.05e-2 which lands
# at ~6.7e-3 end-to-end with 7/16 of k-blocks routed to the DVE.
EC2, EC1, EC0 = 3.03313468e-05, 7.90702397e-03, 1.00029378e+00


def _register_exp16():
    """Register the custom DVE op (documented extension point in dve_ops)."""
    if "EXP16_ANT" in dve_ops._SUB_OPCODE_FOR_NAME:
        return next(o for o in dve_ops.OPS if o.name == "EXP16_ANT")
    body = sq(sq(sq(sq((Src0 * C0 + C1) * Src0 + C2))))

    def ref(in0, in1, s0, s1, imm2):
        p = (in0.astype(np.float32) * s0 + s1) * in0 + imm2
        for _ in range(4):
            p = p * p
        return p

    spec = Spec(body=body, reference=ref)
    shas = {}
    for ver in ("v3", "v4"):
        uops = dve_lower(spec, ver=ver)
        shas[ver] = DveOpSpec(name="EXP16_ANT", opcode=0, uops=uops, rd1_en=False).sha(ver)
    op = DveOp("EXP16_ANT", spec, subdim=False, uops_sha=shas)
    dve_ops.OPS.append(op)
    dve_ops.CUSTOM_DVE_SPECS[op.name] = spec
    dve_ops._SUB_OPCODE_FOR_NAME[op.name] = (
        dve_ops._CUSTOM_DVE_ROW_BASE + len(dve_ops.OPS) - 1
    )
    return op


EXP16 = _register_exp16()


def _emit(tc, nc, xT_d, w_qkvT, b_qkv, w_outT, b_out, y, n):
    NB = n // 128   # token blocks
    QC = n // 512   # q chunks
    KB = n // 128   # k blocks
    EC = E // 128   # e chunks

    persist_cm = tc.tile_pool(name="persist", bufs=1)
    persist = persist_cm.__enter__()

    ones_col = persist.tile([128, 64], BF16, tag="ones_col", name="ones_col")
    nc.vector.memset(ones_col, 1.0)
    ones_row = persist.tile([1, 128], BF16, tag="ones_row", name="ones_row")
    nc.vector.memset(ones_row, 1.0)

    # biases: b_qkv[0:1024] per-partition [128, fb]; v-bias folded into the
    # output-projection bias (sum(softmax)=1): bo2 = b_out + bv @ WoutT
    bqk = persist.tile([128, 8], F32, tag="bqk", name="bqk")
    nc.gpsimd.dma_start(out=bqk, in_=b_qkv[0:1024].rearrange("(a b) -> b a", b=128))
    bv_col = persist.tile([128, 4], F32, tag="bv_col", name="bv_col")
    nc.gpsimd.dma_start(
        out=bv_col, in_=b_qkv[1024:1536].rearrange("(a b) -> b a", b=128)
    )
    bvb = persist.tile([128, 4], BF16, tag="bvb", name="bvb")
    nc.vector.tensor_copy(bvb, bv_col)
    bo_f = persist.tile([1, 512], F32, tag="bo_f", name="bo_f")
    nc.gpsimd.dma_start(out=bo_f, in_=b_out.unsqueeze(0))
    bo2 = persist.tile([1, 512], BF16, tag="bo2", name="bo2")
    bo2b = persist.tile([128, 512], F32, tag="bo2b", name="bo2b")
    wu_sb = persist.tile([128, 512], BF16, tag="wu_sb", name="wu_sb")
    nc.vector.memset(wu_sb, 0.0)

    # persistent bf16 operands (single tiles; fewer tags = smaller exit
    # barrier at the kernel tail)
    xT = persist.tile([128, EC, n], BF16, tag="xT", name="xT")
    wqkvT = persist.tile([128, EC, 1536], BF16, tag="wqkvT", name="wqkvT")
    woutT = persist.tile([128, EC, 512], BF16, tag="woutT", name="woutT")
    qkT = persist.tile([128, 8, n], BF16, tag="qkT", name="qkT")
    v_sb = persist.tile([128, NB, 512], BF16, tag="v_sb", name="v_sb")
    outT = persist.tile([128, 4, n], BF16, tag="outT", name="outT")

    # ---------------- phase 0: load (bf16, pre-transposed on host) + QKV ----
    with (
        tc.tile_pool(name="pqkv", bufs=4, space="PSUM") as pqkv_pool,
    ):
        # Critical-path loads first: the very first matmul only needs the
        # j=0 slivers of W-fb0 and x-ncol0, so those are their own tiny DMAs.
        # Then the j-rest of each, the full-width W lines (split across the
        # sync and gpsimd queues so they land before the v matmuls), the
        # remaining x chunks, and W_out last.
        wq_r = w_qkvT.rearrange("(j p) c -> p j c", p=128)
        x_r = xT_d.rearrange("(j p) c -> p j c", p=128)
        # PE warm-up during the DMA dead time: the HAM clock gate opens after
        # ~3.4us of sustained PE activity, so spin a few matmuls on memset
        # data and the first real matmuls run at 2.4GHz instead of 1.2.
        for wi in range(4):
            pw = pqkv_pool.tile([128, 512], F32, tag="qk" if wi % 2 else "v",
                                name="pw")
            nc.tensor.matmul(pw[0:64, :], lhsT=ones_col, rhs=wu_sb,
                             start=True, stop=True)
        # DMA order matches consumption: qk(0) [W fb0 + x], v [W 1024:1536],
        # qk(4..) [W 128:1024], each j=0 sliver first so the opening matmul
        # of each group never waits a full-width line.
        nc.sync.dma_start(out=wqkvT[:, 0:1, 0:128], in_=wq_r[:, 0:1, 0:128])
        nc.scalar.dma_start(out=xT[:, 0:1, 0:512], in_=x_r[:, 0:1, 0:512])
        nc.sync.dma_start(out=wqkvT[:, 1:EC, 0:128], in_=wq_r[:, 1:EC, 0:128])
        nc.scalar.dma_start(out=xT[:, 1:EC, 0:512], in_=x_r[:, 1:EC, 0:512])
        for j in range(EC):
            nc.sync.dma_start(
                out=wqkvT[:, j, 1024:1536], in_=w_qkvT[j * 128:(j + 1) * 128, 1024:1536]
            )
        for j in range(EC):
            nc.sync.dma_start(
                out=wqkvT[:, j, 128:1024], in_=w_qkvT[j * 128:(j + 1) * 128, 128:1024]
            )
        for ncol in range(1, QC):
            nc.scalar.dma_start(
                out=xT[:, :, ncol * 512:(ncol + 1) * 512],
                in_=x_r[:, :, ncol * 512:(ncol + 1) * 512],
            )
        for j in range(EC):
            nc.gpsimd.dma_start(
                out=woutT[:, j, :], in_=w_outT[j * 128:(j + 1) * 128, :]
            )

        def emit_qk_ncol(fb, ncol, pq=None):
            if pq is None:
                pq = pqkv_pool.tile([128, 512], F32, tag="qk", name="pq")
            for j in range(EC):
                nc.tensor.matmul(
                    pq,
                    lhsT=wqkvT[:, j, fb * 128:(fb + 1) * 128],
                    rhs=xT[:, j, ncol * 512:(ncol + 1) * 512],
                    start=(j == 0),
                    stop=(j == EC - 1),
                )
            nc.vector.tensor_scalar_add(
                qkT[:, fb, ncol * 512:(ncol + 1) * 512], pq, bqk[:, fb:fb + 1]
            )

        def emit_qk(fb):
            for ncol in range(QC):
                emit_qk_ncol(fb, ncol)

        def emit_v(nb):
            pv = pqkv_pool.tile([128, 512], F32, tag="v", name="pv")
            for j in range(EC):
                nc.tensor.matmul(
                    pv,
                    lhsT=xT[:, j, nb * 128:(nb + 1) * 128],
                    rhs=wqkvT[:, j, 1024:1536],
                    start=(j == 0),
                    stop=(j == EC - 1),
                )
            nc.vector.tensor_copy(v_sb[:, nb, :], pv)

        emit_qk(0)
        for nb in range(NB):
            emit_v(nb)
        emit_qk(4)
        for fb in (1, 5, 2, 6, 3):
            emit_qk(fb)
        for ncol in (2, 3):
            emit_qk_ncol(7, ncol)

        # bo2 = b_out + bv @ WoutT (one-time; replaces the separate v-bias).
        # Emitted last so its woutT dependency never blocks the qk/v stream.
        pb = pqkv_pool.tile([128, 512], F32, tag="qk", name="pb")
        for j in range(EC):
            nc.tensor.matmul(
                pb[0:1, :], lhsT=bvb[:, j:j + 1], rhs=woutT[:, j, :],
                start=(j == 0), stop=(j == EC - 1),
            )
        nc.vector.tensor_add(bo2, bo_f, pb[0:1, :])
        # broadcast bo2 over 128 partitions once: the finals then add it on
        # the DVE copy instead of spending a PE slot per block on a rank-1
        # ones-lhsT matmul
        pbb = pqkv_pool.tile([128, 512], F32, tag="qk", name="pbb")
        nc.tensor.matmul(pbb, lhsT=ones_row, rhs=bo2, start=True, stop=True)
        nc.vector.tensor_copy(bo2b, pbb)

    # ---------------- phase 1: attention ----------------
    # 2-kb cycles: each cycle computes two kb's score pairs and (one cycle
    # deferred) their av/pd matmuls - 6 pair-slots ~ 1.29us of PE work - and
    # issues exactly one exp per engine (ScalarE 1.11us, DVE 1.22us), so
    # neither engine ever backlogs.  Cycle 0 of each qc is [ACT,ACT]: the
    # DVE gap there absorbs the previous qc's recip+mul, so the po/pd WAR
    # at the qc boundary resolves before av(kb0) needs the banks.  Score
    # streams rotate by GLOBAL kb (gkb%3): the stream a new qc's kb0 reuses
    # was exp'd 3 kb earlier, not at the previous qc's end.
    STAG = ("sA", "sB", "sC")
    cycles = [tuple(range(s, s + 2)) for s in range(0, KB, 2)]
    with (
        tc.tile_pool(name="ps", bufs=1, space="PSUM") as s_pool,
        tc.tile_pool(name="po", bufs=1, space="PSUM") as o_pool,
        tc.tile_pool(name="se", bufs=3) as e_pool,
        tc.tile_pool(name="sr", bufs=2) as r_pool,
        tc.tile_pool(name="sy", bufs=4) as y_pool,
    ):
        def emit_final(nb, ftag):
            pf = o_pool.tile([128, 512], F32, tag=ftag, name="pf")
            for pp in range(4):
                nc.tensor.matmul(
                    pf, lhsT=outT[:, pp, nb * 128:(nb + 1) * 128],
                    rhs=woutT[:, pp, :], start=(pp == 0), stop=(pp == 3),
                )
            ys = y_pool.tile([128, 512], F32, tag="y", name="ys")
            nc.vector.tensor_add(ys, pf, bo2b)
            nc.sync.dma_start(out=y[nb * 128:(nb + 1) * 128, :], in_=ys)

        # deferred-work queue: (weight in PE pair-slots, closure).  Each
        # cycle flushes ~one cycle's worth so avs trail their exps by one
        # cycle and the finals spread instead of bunching.
        work = []

        def flush(budget=5):
            spent = 0
            while work and spent < budget:
                wt, w = work.pop(0)
                w()
                spent += wt

        # all of qk(7) fills the first qc's pipeline-fill bubble (the PE has
        # no deferred avs in its first cycles, and an idle gap there lets the
        # HAM clock gate re-throttle); its PSUM rides the o/d banks, whose
        # first real write av(kb0) is emitted after these flush
        def qk7_fill(ncol, ftag):
            pq = o_pool.tile([128, 512], F32, tag=ftag, name="pq7")
            emit_qk_ncol(7, ncol, pq=pq)

        for ncol in range(2):
            work.append((4, lambda ncol=ncol, t=("o" if ncol % 2 == 0 else "d"):
                         qk7_fill(ncol, t)))

        for p in range(4):
            for qc in range(QC):
                qs = slice(qc * 512, (qc + 1) * 512)
                po = o_pool.tile([128, 512], F32, tag="o", name="po")
                pd = o_pool.tile([128, 512], F32, tag="d", name="pd")

                def av(e, kb, po=po, pd=pd, p=p):
                    first, last = (kb == 0), (kb == KB - 1)
                    eA = e[:, 0, :]
                    eB = e[:, 1, :]

                    def po_mm():
                        nc.tensor.matmul(
                            po[0:64, :], lhsT=v_sb[:, kb, p * 128:p * 128 + 64],
                            rhs=eA, start=first, stop=last, skip_group_check=True,
                        )
                        nc.tensor.matmul(
                            po[64:128, :],
                            lhsT=v_sb[:, kb, p * 128 + 64:(p + 1) * 128],
                            rhs=eB, start=first, stop=last, skip_group_check=True,
                        )

                    def pd_mm():
                        nc.tensor.matmul(
                            pd[0:64, :], lhsT=ones_col, rhs=eA,
                            start=first, stop=last, skip_group_check=True,
                        )
                        nc.tensor.matmul(
                            pd[64:128, :], lhsT=ones_col, rhs=eB,
                            start=first, stop=last, skip_group_check=True,
                        )

                    # on the last k-block, finish pd first so the reciprocal
                    # in the epilogue starts two slots earlier
                    if last:
                        pd_mm(); po_mm()
                    else:
                        po_mm(); pd_mm()

                def normalize(po=po, pd=pd, p=p, qs=qs):
                    rc = r_pool.tile([128, 512], F32, tag="rc", name="rc")
                    nc.vector.reciprocal_approx_fast(rc, pd)
                    nc.vector.tensor_mul(outT[:, p, qs], po, rc)

                for ci, cyc in enumerate(cycles):
                    new_avs = []
                    for i, kb in enumerate(cyc):
                        gkb = ((p * QC + qc) * KB + kb)
                        st = STAG[gkb % 3]
                        ks = slice(kb * 128, (kb + 1) * 128)
                        S = s_pool.tile([128, 2, 512], F32, tag=st, name="S")
                        nc.tensor.matmul(
                            S[:, 0, :], lhsT=qkT[0:64, 4 + p, ks],
                            rhs=qkT[0:64, p, qs], start=True, stop=True,
                        )
                        nc.tensor.matmul(
                            S[:, 1, :], lhsT=qkT[64:128, 4 + p, ks],
                            rhs=qkT[64:128, p, qs], start=True, stop=True,
                        )
                        e = e_pool.tile([128, 2, 512], BF16, tag="e" + st, name="e")
                        if ci == 0:
                            on_dve = False          # [A, A] boundary cycle
                        elif ci % 2 == 1:
                            on_dve = (i == 0)       # [D, A]
                        else:
                            on_dve = (i == 1)       # [A, D]
                        if on_dve:
                            nc.vector._custom_dve(
                                EXP16, out=e, in0=S, s0=EC2, s1=EC1, imm2=EC0
                            )
                        else:
                            nc.scalar.activation(
                                e, S, mybir.ActivationFunctionType.Exp, scale=0.125,
                            )
                        new_avs.append(
                            (2, lambda e=e, kb=kb, av=av: av(e, kb))
                        )
                    flush()
                    work.extend(new_avs)
                # normalization and (on the last pair) the finished output
                # rows join the deferred queue so the next qc's scores/exps
                # stay ahead of them
                work.append((0, normalize))
                if p == 3:
                    for i, nb in enumerate(range(qc * 4, qc * 4 + 4)):
                        work.append(
                            (4, lambda nb=nb, t=("o" if i % 2 == 0 else "d"),
                                emit_final=emit_final: emit_final(nb, t))
                        )
        while work:
            flush()
    persist_cm.__exit__(None, None, None)


def build(n=N_SEQ):
    nc = bacc.Bacc("TRN2", target_bir_lowering=False, debug=False)
    xT_d = nc.dram_tensor("xT", [E, n], BF16, kind="ExternalInput").ap()
    w_qkvT = nc.dram_tensor("w_qkvT", [E, 3 * E], BF16, kind="ExternalInput").ap()
    b_qkv = nc.dram_tensor("b_qkv", [3 * E], F32, kind="ExternalInput").ap()
    w_outT = nc.dram_tensor("w_outT", [E, E], BF16, kind="ExternalInput").ap()
    b_out = nc.dram_tensor("b_out", [E], F32, kind="ExternalInput").ap()
    y = nc.dram_tensor("y", [n, E], F32, kind="ExternalOutput").ap()
    with tile.TileContext(nc) as tc:
        _emit(tc, nc, xT_d, w_qkvT, b_qkv, w_outT, b_out, y, n)
    nc.compile()
    return nc


_NC_CACHE = {}


def _get_nc(n):
    if n not in _NC_CACHE:
        _NC_CACHE[n] = build(n)
    return _NC_CACHE[n]


def _in_maps(seq, W_qkv, b_qkv, W_out, b_out):
    import ml_dtypes

    bf16 = ml_dtypes.bfloat16
    seq = np.asarray(seq, np.float32)
    wqT = np.ascontiguousarray(np.asarray(W_qkv, np.float32).T.astype(bf16))
    bq = np.ascontiguousarray(np.asarray(b_qkv, np.float32))
    woT = np.ascontiguousarray(np.asarray(W_out, np.float32).T.astype(bf16))
    bo = np.ascontiguousarray(np.asarray(b_out, np.float32))
    return [
        {
            "xT": np.ascontiguousarray(seq[:, b, :].T.astype(bf16)),  # [E, n]
            "w_qkvT": wqT,
            "b_qkv": bq,
            "w_outT": woT,
            "b_out": bo,
        }
        for b in range(seq.shape[1])
    ]


def run(seq, W_qkv, b_qkv, W_out, b_out, trace=False):
    """Returns (out [n, bs, e] fp32, BassKernelResults)."""
    from concourse.bass_utils import run_bass_kernel_spmd

    seq = np.asarray(seq, np.float32)
    n, bs, e = seq.shape
    nc = _get_nc(n)
    res = run_bass_kernel_spmd(
        nc,
        _in_maps(seq, W_qkv, b_qkv, W_out, b_out),
        core_ids=list(range(N_CORES)),
        trace=trace,
    )
    out = np.empty((n, bs, e), np.float32)
    for b in range(bs):
        out[:, b, :] = res.results[b]["y"]
    return out, res


def kernel(seq, W_qkv, b_qkv, W_out, b_out):
    out, _ = run(seq, W_qkv, b_qkv, W_out, b_out)
    return out


# revision 39
# speedup vs baseline: 1.0119x; 1.0018x over previous
"""Multi-head self-attention Trainium2 kernel (Bass/Tile), batch-sharded SPMD.

Problem: seq [2048, 8, 512] fp32, fused QKV (W_qkv [1536,512], b_qkv [1536]),
H=8 heads of HD=64, full softmax attention, out proj (W_out [512,512], b_out).

Sharding: batch (bs=8) across 8 NeuronCores, one batch element per core, no
collectives. The host pre-transposes per-core x -> xT [e, n] and the weights
(and casts them to bf16), scatters, and gathers y -> [n, bs, e].

Per-core dataflow (n=2048, E=512, all matmuls bf16 with fp32 PSUM):
  qkT [f, n] <- WqkvT.T @ xT   (f in [0,1024): q|k features; each 128-row
                tile holds a head PAIR: rows 0:64 head 2p, 64:128 head 2p+1)
  v   [n, f] <- xT.T @ WvT     (no bias matmul: since sum(softmax)=1, the
                v-bias is folded into the out-proj bias bo2 = b_out+bv@WoutT)
  attention, per head pair p, per q-chunk (512 cols), 3-kb cycles:
    scoresT[k,q]: row-paired matmuls into per-kb streams sA/sB/sC
    exp: balanced across ScalarE (exact ACTIVATE) and DVE (custom EXP16_ANT,
         exp(s/8) ~ poly^16) at 9:7 per qc - the v1 split (ACT ~2.2us/cycle
         vs the 1.94us PE cycle) made ACT the wall and stalled each cycle
    av/denominator (deferred one cycle): col-paired matmuls po += v.T @ e,
        pd += ones.T @ e; epilogue rc=1/pd on DVE, outT = po*rc on DVE
  y [n, f] = outT.T @ WoutT + b: bias enters as a K=1 ones_row x bo2 matmul
    into PSUM, so the epilogue is a pure PSUM->SBUF copy split ScalarE/DVE.

Changes vs the 302.7us prior version (trace-driven; measured 297.0us,
rel err 7.0e-3):
  - 2-kb cycles (was 3): each cycle carries exactly one ScalarE exp and one
    DVE exp (1114/1224ns, both under the ~1.3us PE cycle), killing the
    per-cycle integer imbalance (2 ACT exps = 2.23us/cycle vs a 1.94us
    3-kb PE cycle) that stalled the PE ~700-900ns every cycle
  - cycle 0 of each qc is [ACT,ACT]: the DVE gap absorbs the previous qc's
    recip+mul so the po/pd WAR at the qc boundary stops stalling the PE
  - score streams rotate by GLOBAL kb index (gkb%3) so the first stream a
    new qc reuses was exp'd 3 kb earlier, not at the previous qc's end
  - exp split 9:7 ScalarE:DVE per qc (engine-balanced incl. normalize)
  - startup: j=0 slivers of W-fb0/x-ncol0 land first; W ordered fb0 -> v
    slice -> the rest to match qk(0) -> v -> qk(4..) consumption; PE warm-up
    matmuls during the DMA dead time so the HAM clock gate opens early
  - qk(7) n0/n1 ride the attention deferred-work queue (their PSUM borrows
    the o/d banks) to fill the first qc's exp pipeline-fill bubble
  - persistent tiles consolidated (qkT/v/outT single tiles): smaller exit
    barrier at the tail

Where the remaining time goes (per the NTFF profile): PE streaming floor
~218us + ~30us exposed LDWEIGHTS at paired-matmul quadrant transitions
(tile_position'd loads cannot use the background weight buffer) + ~47us
projection phase + ~6us fixed runtime setup + ~6us tail teardown.  The
attention phase runs cycle-lockstep (3 single-buffered score streams), so
per-cycle time = max(PE, exp engine) every cycle; PE and the two exp
engines are co-bound within ~10%.  Rejected with numbers: fp8/DoubleRow
anywhere in the value path (~2.5% rel err - quantization of a random
weighted sum passes through 1:1, gate is 2e-2), GpSimd softmax-denominator
offload (partition_all_reduce 13.2us per [128,2048]; tensor_add 2.1us per
[128,1024] - capacity-dead), and e-pair pre-summing for pd (PE savings in
light cycles don't transfer across the lockstep; measured neutral).
"""

import numpy as np

import concourse.bass as bass
import concourse.mybir as mybir
import concourse.tile as tile
from concourse import bacc
from concourse import dve_ops
from concourse.dve_spec import Spec, Src0, C0, C1, C2, sq
from concourse.dve_uop import DveOpSpec
from concourse.dve_ops import DveOp
from concourse.dve_spec import lower as dve_lower

F32 = mybir.dt.float32
BF16 = mybir.dt.bfloat16

N_SEQ, BS, E, H, HD = 2048, 8, 512, 8, 64
N_CORES = 8

# exp(0.125*s) ~ ((EC2*s + EC1)*s + EC0)^16, minimax-fitted on s in [-40, 40]
# (observed raw-score range is [-36.3, 37.2]); max rel err 2.05e-2 which lands
# at ~6.7e-3 end-to-end with 7/16 of k-blocks routed to the DVE.
EC2, EC1, EC0 = 3.03313468e-05, 7.90702397e-03, 1.00029378e+00


def _register_exp16():
    """Register the custom DVE op (documented extension point in dve_ops)."""
    if "EXP16_ANT" in dve_ops._SUB_OPCODE_FOR_NAME:
        return next(o for o in dve_ops.OPS if o.name == "EXP16_ANT")
    body = sq(sq(sq(sq((Src0 * C0 + C1) * Src0 + C2))))

    def ref(in0, in1, s0, s1, imm2):
        p = (in0.astype(np.float32) * s0 + s1) * in0 + imm2
        for _ in range(4):
            p = p * p
        return p

    spec = Spec(body=body, reference=ref)
    shas = {}
    for ver in ("v3", "v4"):
        uops = dve_lower(spec, ver=ver)
        shas[ver] = DveOpSpec(name="EXP16_ANT", opcode=0, uops=uops, rd1_en=False).sha(ver)
    op = DveOp("EXP16_ANT", spec, subdim=False, uops_sha=shas)
    dve_ops.OPS.append(op)
    dve_ops.CUSTOM_DVE_SPECS[op.name] = spec
    dve_ops._SUB_OPCODE_FOR_NAME[op.name] = (
        dve_ops._CUSTOM_DVE_ROW_BASE + len(dve_ops.OPS) - 1
    )
    return op


EXP16 = _register_exp16()


def _emit(tc, nc, xT_d, w_qkvT, b_qkv, w_outT, b_out, y, n):
    NB = n // 128   # token blocks
    QC = n // 512   # q chunks
    KB = n // 128   # k blocks
    EC = E // 128   # e chunks

    persist_cm = tc.tile_pool(name="persist", bufs=1)
    persist = persist_cm.__enter__()

    ones_col = persist.tile([128, 64], BF16, tag="ones_col", name="ones_col")
    nc.vector.memset(ones_col, 1.0)
    ones_row = persist.tile([1, 128], BF16, tag="ones_row", name="ones_row")
    nc.vector.memset(ones_row, 1.0)

    # biases: b_qkv[0:1024] per-partition [128, fb]; v-bias folded into the
    # output-projection bias (sum(softmax)=1): bo2 = b_out + bv @ WoutT
    bqk = persist.tile([128, 8], F32, tag="bqk", name="bqk")
    nc.gpsimd.dma_start(out=bqk, in_=b_qkv[0:1024].rearrange("(a b) -> b a", b=128))
    bv_col = persist.tile([128, 4], F32, tag="bv_col", name="bv_col")
    nc.gpsimd.dma_start(
        out=bv_col, in_=b_qkv[1024:1536].rearrange("(a b) -> b a", b=128)
    )
    bvb = persist.tile([128, 4], BF16, tag="bvb", name="bvb")
    nc.vector.tensor_copy(bvb, bv_col)
    bo_f = persist.tile([1, 512], F32, tag="bo_f", name="bo_f")
    nc.gpsimd.dma_start(out=bo_f, in_=b_out.unsqueeze(0))
    bo2 = persist.tile([1, 512], BF16, tag="bo2", name="bo2")
    bo2b = persist.tile([128, 512], F32, tag="bo2b", name="bo2b")
    wu_sb = persist.tile([128, 512], BF16, tag="wu_sb", name="wu_sb")
    nc.vector.memset(wu_sb, 0.0)

    # persistent bf16 operands (single tiles; fewer tags = smaller exit
    # barrier at the kernel tail)
    xT = persist.tile([128, EC, n], BF16, tag="xT", name="xT")
    wqkvT = persist.tile([128, EC, 1536], BF16, tag="wqkvT", name="wqkvT")
    woutT = persist.tile([128, EC, 512], BF16, tag="woutT", name="woutT")
    qkT = persist.tile([128, 8, n], BF16, tag="qkT", name="qkT")
    v_sb = persist.tile([128, NB, 512], BF16, tag="v_sb", name="v_sb")
    outT = persist.tile([128, 4, n], BF16, tag="outT", name="outT")

    # ---------------- phase 0: load (bf16, pre-transposed on host) + QKV ----
    with (
        tc.tile_pool(name="pqkv", bufs=4, space="PSUM") as pqkv_pool,
    ):
        # Critical-path loads first: the very first matmul only needs the
        # j=0 slivers of W-fb0 and x-ncol0, so those are their own tiny DMAs.
        # Then the j-rest of each, the full-width W lines (split across the
        # sync and gpsimd queues so they land before the v matmuls), the
        # remaining x chunks, and W_out last.
        wq_r = w_qkvT.rearrange("(j p) c -> p j c", p=128)
        x_r = xT_d.rearrange("(j p) c -> p j c", p=128)
        # PE warm-up during the DMA dead time: the HAM clock gate opens after
        # ~3.4us of sustained PE activity, so spin a few matmuls on memset
        # data and the first real matmuls run at 2.4GHz instead of 1.2.
        for wi in range(4):
            pw = pqkv_pool.tile([128, 512], F32, tag="qk" if wi % 2 else "v",
                                name="pw")
            nc.tensor.matmul(pw[0:64, :], lhsT=ones_col, rhs=wu_sb,
                             start=True, stop=True)
        # DMA order matches consumption: qk(0) [W fb0 + x], v [W 1024:1536],
        # qk(4..) [W 128:1024], each j=0 sliver first so the opening matmul
        # of each group never waits a full-width line.
        nc.sync.dma_start(out=wqkvT[:, 0:1, 0:128], in_=wq_r[:, 0:1, 0:128])
        nc.scalar.dma_start(out=xT[:, 0:1, 0:512], in_=x_r[:, 0:1, 0:512])
        nc.sync.dma_start(out=wqkvT[:, 1:EC, 0:128], in_=wq_r[:, 1:EC, 0:128])
        nc.scalar.dma_start(out=xT[:, 1:EC, 0:512], in_=x_r[:, 1:EC, 0:512])
        for j in range(EC):
            nc.sync.dma_start(
                out=wqkvT[:, j, 1024:1536], in_=w_qkvT[j * 128:(j + 1) * 128, 1024:1536]
            )
        for j in range(EC):
            nc.sync.dma_start(
                out=wqkvT[:, j, 128:1024], in_=w_qkvT[j * 128:(j + 1) * 128, 128:1024]
            )
        for ncol in range(1, QC):
            nc.scalar.dma_start(
                out=xT[:, :, ncol * 512:(ncol + 1) * 512],
                in_=x_r[:, :, ncol * 512:(ncol + 1) * 512],
            )
        for j in range(EC):
            nc.gpsimd.dma_start(
                out=woutT[:, j, :], in_=w_outT[j * 128:(j + 1) * 128, :]
            )

        def emit_qk_ncol(fb, ncol, pq=None):
            if pq is None:
                pq = pqkv_pool.tile([128, 512], F32, tag="qk", name="pq")
            for j in range(EC):
                nc.tensor.matmul(
                    pq,
                    lhsT=wqkvT[:, j, fb * 128:(fb + 1) * 128],
                    rhs=xT[:, j, ncol * 512:(ncol + 1) * 512],
                    start=(j == 0),
                    stop=(j == EC - 1),
                )
            nc.vector.tensor_scalar_add(
                qkT[:, fb, ncol * 512:(ncol + 1) * 512], pq, bqk[:, fb:fb + 1]
            )

        def emit_qk(fb):
            for ncol in range(QC):
                emit_qk_ncol(fb, ncol)

        def emit_v(nb):
            pv = pqkv_pool.tile([128, 512], F32, tag="v", name="pv")
            for j in range(EC):
                nc.tensor.matmul(
                    pv,
                    lhsT=xT[:, j, nb * 128:(nb + 1) * 128],
                    rhs=wqkvT[:, j, 1024:1536],
                    start=(j == 0),
                    stop=(j == EC - 1),
                )
            nc.vector.tensor_copy(v_sb[:, nb, :], pv)

        emit_qk(0)
        for nb in range(NB):
            emit_v(nb)
        emit_qk(4)
        for fb in (1, 5, 2, 6, 3):
            emit_qk(fb)
        for ncol in (2, 3):
            emit_qk_ncol(7, ncol)

        # bo2 = b_out + bv @ WoutT (one-time; replaces the separate v-bias).
        # Emitted last so its woutT dependency never blocks the qk/v stream.
        pb = pqkv_pool.tile([128, 512], F32, tag="qk", name="pb")
        for j in range(EC):
            nc.tensor.matmul(
                pb[0:1, :], lhsT=bvb[:, j:j + 1], rhs=woutT[:, j, :],
                start=(j == 0), stop=(j == EC - 1),
            )
        nc.vector.tensor_add(bo2, bo_f, pb[0:1, :])
        # broadcast bo2 over 128 partitions once: the finals then add it on
        # the DVE copy instead of spending a PE slot per block on a rank-1
        # ones-lhsT matmul
        pbb = pqkv_pool.tile([128, 512], F32, tag="qk", name="pbb")
        nc.tensor.matmul(pbb, lhsT=ones_row, rhs=bo2, start=True, stop=True)
        nc.vector.tensor_copy(bo2b, pbb)

    # ---------------- phase 1: attention ----------------
    # 2-kb cycles: each cycle computes two kb's score pairs and (one cycle
    # deferred) their av/pd matmuls - 6 pair-slots ~ 1.29us of PE work - and
    # issues exactly one exp per engine (ScalarE 1.11us, DVE 1.22us), so
    # neither engine ever backlogs.  Cycle 0 of each qc is [ACT,ACT]: the
    # DVE gap there absorbs the previous qc's recip+mul, so the po/pd WAR
    # at the qc boundary resolves before av(kb0) needs the banks.  Score
    # streams rotate by GLOBAL kb (gkb%3): the stream a new qc's kb0 reuses
    # was exp'd 3 kb earlier, not at the previous qc's end.
    STAG = ("sA", "sB", "sC")
    cycles = [tuple(range(s, s + 2)) for s in range(0, KB, 2)]
    with (
        tc.tile_pool(name="ps", bufs=1, space="PSUM") as s_pool,
        tc.tile_pool(name="po", bufs=1, space="PSUM") as o_pool,
        tc.tile_pool(name="se", bufs=3) as e_pool,
        tc.tile_pool(name="sr", bufs=2) as r_pool,
        tc.tile_pool(name="sy", bufs=4) as y_pool,
    ):
        def emit_final(nb, ftag):
            pf = o_pool.tile([128, 512], F32, tag=ftag, name="pf")
            for pp in range(4):
                nc.tensor.matmul(
                    pf, lhsT=outT[:, pp, nb * 128:(nb + 1) * 128],
                    rhs=woutT[:, pp, :], start=(pp == 0), stop=(pp == 3),
                )
            ys = y_pool.tile([128, 512], F32, tag="y", name="ys")
            nc.vector.tensor_add(ys, pf, bo2b)
            nc.sync.dma_start(out=y[nb * 128:(nb + 1) * 128, :], in_=ys)

        # deferred-work queue: (weight in PE pair-slots, closure).  Each
        # cycle flushes ~one cycle's worth so avs trail their exps by one
        # cycle and the finals spread instead of bunching.
        work = []

        def flush(budget=5):
            spent = 0
            while work and spent < budget:
                wt, w = work.pop(0)
                w()
                spent += wt

        # all of qk(7) fills the first qc's pipeline-fill bubble (the PE has
        # no deferred avs in its first cycles, and an idle gap there lets the
        # HAM clock gate re-throttle); its PSUM rides the o/d banks, whose
        # first real write av(kb0) is emitted after these flush
        def qk7_fill(ncol, ftag):
            pq = o_pool.tile([128, 512], F32, tag=ftag, name="pq7")
            emit_qk_ncol(7, ncol, pq=pq)

        for ncol in range(2):
            work.append((4, lambda ncol=ncol, t=("o" if ncol % 2 == 0 else "d"):
                         qk7_fill(ncol, t)))

        for p in range(4):
            for qc in range(QC):
                qs = slice(qc * 512, (qc + 1) * 512)
                po = o_pool.tile([128, 512], F32, tag="o", name="po")
                pd = o_pool.tile([128, 512], F32, tag="d", name="pd")

                def po_one(e, kb, po=po, p=p):
                    first, last = (kb == 0), (kb == KB - 1)
                    nc.tensor.matmul(
                        po[0:64, :], lhsT=v_sb[:, kb, p * 128:p * 128 + 64],
                        rhs=e[:, 0, :], start=first, stop=last,
                        skip_group_check=True,
                    )
                    nc.tensor.matmul(
                        po[64:128, :],
                        lhsT=v_sb[:, kb, p * 128 + 64:(p + 1) * 128],
                        rhs=e[:, 1, :], start=first, stop=last,
                        skip_group_check=True,
                    )

                def pd_one(e, kb, pd=pd):
                    first, last = (kb == 0), (kb == KB - 1)
                    nc.tensor.matmul(
                        pd[0:64, :], lhsT=ones_col, rhs=e[:, 0, :],
                        start=first, stop=last, skip_group_check=True,
                    )
                    nc.tensor.matmul(
                        pd[64:128, :], lhsT=ones_col, rhs=e[:, 1, :],
                        start=first, stop=last, skip_group_check=True,
                    )

                def av2(t_i, t_j, po_one=po_one, pd_one=pd_one):
                    # grouped po,po,pd,pd: the two pd pairs load IDENTICAL
                    # ones weights back-to-back (dedupe/prefetch friendly).
                    # On the last k-block run the pds first so the epilogue
                    # reciprocal starts two slots earlier.
                    (e_i, kb_i), (e_j, kb_j) = t_i, t_j
                    if kb_j == KB - 1:
                        pd_one(e_i, kb_i); pd_one(e_j, kb_j)
                        po_one(e_i, kb_i); po_one(e_j, kb_j)
                    else:
                        po_one(e_i, kb_i); po_one(e_j, kb_j)
                        pd_one(e_i, kb_i); pd_one(e_j, kb_j)

                def normalize(po=po, pd=pd, p=p, qs=qs):
                    rc = r_pool.tile([128, 512], F32, tag="rc", name="rc")
                    nc.vector.reciprocal_approx_fast(rc, pd)
                    nc.vector.tensor_mul(outT[:, p, qs], po, rc)

                for ci, cyc in enumerate(cycles):
                    tiles = []
                    for i, kb in enumerate(cyc):
                        gkb = ((p * QC + qc) * KB + kb)
                        st = STAG[gkb % 3]
                        ks = slice(kb * 128, (kb + 1) * 128)
                        S = s_pool.tile([128, 2, 512], F32, tag=st, name="S")
                        nc.tensor.matmul(
                            S[:, 0, :], lhsT=qkT[0:64, 4 + p, ks],
                            rhs=qkT[0:64, p, qs], start=True, stop=True,
                        )
                        nc.tensor.matmul(
                            S[:, 1, :], lhsT=qkT[64:128, 4 + p, ks],
                            rhs=qkT[64:128, p, qs], start=True, stop=True,
                        )
                        e = e_pool.tile([128, 2, 512], BF16, tag="e" + st, name="e")
                        if ci == 0:
                            on_dve = False          # [A, A] boundary cycle
                        elif ci % 2 == 1:
                            on_dve = (i == 0)       # [D, A]
                        else:
                            on_dve = (i == 1)       # [A, D]
                        if on_dve:
                            nc.vector._custom_dve(
                                EXP16, out=e, in0=S, s0=EC2, s1=EC1, imm2=EC0
                            )
                        else:
                            nc.scalar.activation(
                                e, S, mybir.ActivationFunctionType.Exp, scale=0.125,
                            )
                        tiles.append((e, kb))
                    flush()
                    work.append(
                        (4, lambda a=tiles[0], b=tiles[1], f=av2: f(a, b))
                    )
                # normalization and (on the last pair) the finished output
                # rows join the deferred queue so the next qc's scores/exps
                # stay ahead of them
                work.append((0, normalize))
                if p == 3:
                    for i, nb in enumerate(range(qc * 4, qc * 4 + 4)):
                        work.append(
                            (4, lambda nb=nb, t=("o" if i % 2 == 0 else "d"),
                                emit_final=emit_final: emit_final(nb, t))
                        )
        while work:
            flush()
    persist_cm.__exit__(None, None, None)


def build(n=N_SEQ):
    nc = bacc.Bacc("TRN2", target_bir_lowering=False, debug=False)
    xT_d = nc.dram_tensor("xT", [E, n], BF16, kind="ExternalInput").ap()
    w_qkvT = nc.dram_tensor("w_qkvT", [E, 3 * E], BF16, kind="ExternalInput").ap()
    b_qkv = nc.dram_tensor("b_qkv", [3 * E], F32, kind="ExternalInput").ap()
    w_outT = nc.dram_tensor("w_outT", [E, E], BF16, kind="ExternalInput").ap()
    b_out = nc.dram_tensor("b_out", [E], F32, kind="ExternalInput").ap()
    y = nc.dram_tensor("y", [n, E], F32, kind="ExternalOutput").ap()
    with tile.TileContext(nc) as tc:
        _emit(tc, nc, xT_d, w_qkvT, b_qkv, w_outT, b_out, y, n)
    nc.compile()
    return nc


_NC_CACHE = {}


def _get_nc(n):
    if n not in _NC_CACHE:
        _NC_CACHE[n] = build(n)
    return _NC_CACHE[n]


def _in_maps(seq, W_qkv, b_qkv, W_out, b_out):
    import ml_dtypes

    bf16 = ml_dtypes.bfloat16
    seq = np.asarray(seq, np.float32)
    wqT = np.ascontiguousarray(np.asarray(W_qkv, np.float32).T.astype(bf16))
    bq = np.ascontiguousarray(np.asarray(b_qkv, np.float32))
    woT = np.ascontiguousarray(np.asarray(W_out, np.float32).T.astype(bf16))
    bo = np.ascontiguousarray(np.asarray(b_out, np.float32))
    return [
        {
            "xT": np.ascontiguousarray(seq[:, b, :].T.astype(bf16)),  # [E, n]
            "w_qkvT": wqT,
            "b_qkv": bq,
            "w_outT": woT,
            "b_out": bo,
        }
        for b in range(seq.shape[1])
    ]


def run(seq, W_qkv, b_qkv, W_out, b_out, trace=False):
    """Returns (out [n, bs, e] fp32, BassKernelResults)."""
    from concourse.bass_utils import run_bass_kernel_spmd

    seq = np.asarray(seq, np.float32)
    n, bs, e = seq.shape
    nc = _get_nc(n)
    res = run_bass_kernel_spmd(
        nc,
        _in_maps(seq, W_qkv, b_qkv, W_out, b_out),
        core_ids=list(range(N_CORES)),
        trace=trace,
    )
    out = np.empty((n, bs, e), np.float32)
    for b in range(bs):
        out[:, b, :] = res.results[b]["y"]
    return out, res


def kernel(seq, W_qkv, b_qkv, W_out, b_out):
    out, _ = run(seq, W_qkv, b_qkv, W_out, b_out)
    return out


# revision 40
# speedup vs baseline: 1.0123x; 1.0004x over previous
"""Multi-head self-attention Trainium2 kernel (Bass/Tile), batch-sharded SPMD.

Problem: seq [2048, 8, 512] fp32, fused QKV (W_qkv [1536,512], b_qkv [1536]),
H=8 heads of HD=64, full softmax attention, out proj (W_out [512,512], b_out).

Sharding: batch (bs=8) across 8 NeuronCores, one batch element per core, no
collectives. The host pre-transposes per-core x -> xT [e, n] and the weights
(and casts them to bf16), scatters, and gathers y -> [n, bs, e].

Per-core dataflow (n=2048, E=512, all matmuls bf16 with fp32 PSUM):
  qkT [f, n] <- WqkvT.T @ xT   (f in [0,1024): q|k features; each 128-row
                tile holds a head PAIR: rows 0:64 head 2p, 64:128 head 2p+1)
  v   [n, f] <- xT.T @ WvT     (no bias matmul: since sum(softmax)=1, the
                v-bias is folded into the out-proj bias bo2 = b_out+bv@WoutT)
  attention, per head pair p, per q-chunk (512 cols), 3-kb cycles:
    scoresT[k,q]: row-paired matmuls into per-kb streams sA/sB/sC
    exp: balanced across ScalarE (exact ACTIVATE) and DVE (custom EXP16_ANT,
         exp(s/8) ~ poly^16) at 9:7 per qc - the v1 split (ACT ~2.2us/cycle
         vs the 1.94us PE cycle) made ACT the wall and stalled each cycle
    av/denominator (deferred one cycle): col-paired matmuls po += v.T @ e,
        pd += ones.T @ e; epilogue rc=1/pd on DVE, outT = po*rc on DVE
  y [n, f] = outT.T @ WoutT + b: bias enters as a K=1 ones_row x bo2 matmul
    into PSUM, so the epilogue is a pure PSUM->SBUF copy split ScalarE/DVE.

Changes vs the 302.7us prior version (trace-driven; measured 297.0us,
rel err 7.0e-3):
  - 2-kb cycles (was 3): each cycle carries exactly one ScalarE exp and one
    DVE exp (1114/1224ns, both under the ~1.3us PE cycle), killing the
    per-cycle integer imbalance (2 ACT exps = 2.23us/cycle vs a 1.94us
    3-kb PE cycle) that stalled the PE ~700-900ns every cycle
  - cycle 0 of each qc is [ACT,ACT]: the DVE gap absorbs the previous qc's
    recip+mul so the po/pd WAR at the qc boundary stops stalling the PE
  - score streams rotate by GLOBAL kb index (gkb%3) so the first stream a
    new qc reuses was exp'd 3 kb earlier, not at the previous qc's end
  - exp split 9:7 ScalarE:DVE per qc (engine-balanced incl. normalize)
  - startup: j=0 slivers of W-fb0/x-ncol0 land first; W ordered fb0 -> v
    slice -> the rest to match qk(0) -> v -> qk(4..) consumption; PE warm-up
    matmuls during the DMA dead time so the HAM clock gate opens early
  - qk(7) n0/n1 ride the attention deferred-work queue (their PSUM borrows
    the o/d banks) to fill the first qc's exp pipeline-fill bubble
  - av work grouped po,po,pd,pd per 2-kb cycle (fewer worst-case LDWEIGHTS
    quadrant transitions; walrus does NOT dedupe the identical ones-loads)
  - persistent tiles consolidated (qkT/v/outT single tiles): smaller exit
    barrier at the tail

Where the remaining time goes (per the NTFF profile): PE streaming floor
~218us + ~30us exposed LDWEIGHTS at paired-matmul quadrant transitions
(tile_position'd loads cannot use the background weight buffer) + ~47us
projection phase + ~6us fixed runtime setup + ~6us tail teardown.  The
attention phase runs cycle-lockstep (3 single-buffered score streams), so
per-cycle time = max(PE, exp engine) every cycle; PE and the two exp
engines are co-bound within ~10%.  Rejected with numbers: fp8/DoubleRow
anywhere in the value path (~2.5% rel err - quantization of a random
weighted sum passes through 1:1, gate is 2e-2), GpSimd softmax-denominator
offload (partition_all_reduce 13.2us per [128,2048]; tensor_add 2.1us per
[128,1024] - capacity-dead), and e-pair pre-summing for pd (PE savings in
light cycles don't transfer across the lockstep; measured neutral).
"""

import numpy as np

import concourse.bass as bass
import concourse.mybir as mybir
import concourse.tile as tile
from concourse import bacc
from concourse import dve_ops
from concourse.dve_spec import Spec, Src0, C0, C1, C2, sq
from concourse.dve_uop import DveOpSpec
from concourse.dve_ops import DveOp
from concourse.dve_spec import lower as dve_lower

F32 = mybir.dt.float32
BF16 = mybir.dt.bfloat16

N_SEQ, BS, E, H, HD = 2048, 8, 512, 8, 64
N_CORES = 8

# exp(0.125*s) ~ ((EC2*s + EC1)*s + EC0)^16, minimax-fitted on s in [-40, 40]
# (observed raw-score range is [-36.3, 37.2]); max rel err 2.05e-2 which lands
# at ~6.7e-3 end-to-end with 7/16 of k-blocks routed to the DVE.
EC2, EC1, EC0 = 3.03313468e-05, 7.90702397e-03, 1.00029378e+00


def _register_exp16():
    """Register the custom DVE op (documented extension point in dve_ops)."""
    if "EXP16_ANT" in dve_ops._SUB_OPCODE_FOR_NAME:
        return next(o for o in dve_ops.OPS if o.name == "EXP16_ANT")
    body = sq(sq(sq(sq((Src0 * C0 + C1) * Src0 + C2))))

    def ref(in0, in1, s0, s1, imm2):
        p = (in0.astype(np.float32) * s0 + s1) * in0 + imm2
        for _ in range(4):
            p = p * p
        return p

    spec = Spec(body=body, reference=ref)
    shas = {}
    for ver in ("v3", "v4"):
        uops = dve_lower(spec, ver=ver)
        shas[ver] = DveOpSpec(name="EXP16_ANT", opcode=0, uops=uops, rd1_en=False).sha(ver)
    op = DveOp("EXP16_ANT", spec, subdim=False, uops_sha=shas)
    dve_ops.OPS.append(op)
    dve_ops.CUSTOM_DVE_SPECS[op.name] = spec
    dve_ops._SUB_OPCODE_FOR_NAME[op.name] = (
        dve_ops._CUSTOM_DVE_ROW_BASE + len(dve_ops.OPS) - 1
    )
    return op


EXP16 = _register_exp16()


def _emit(tc, nc, xT_d, w_qkvT, b_qkv, w_outT, b_out, y, n):
    NB = n // 128   # token blocks
    QC = n // 512   # q chunks
    KB = n // 128   # k blocks
    EC = E // 128   # e chunks

    persist_cm = tc.tile_pool(name="persist", bufs=1)
    persist = persist_cm.__enter__()

    ones_col = persist.tile([128, 64], BF16, tag="ones_col", name="ones_col")
    nc.vector.memset(ones_col, 1.0)
    ones_row = persist.tile([1, 128], BF16, tag="ones_row", name="ones_row")
    nc.vector.memset(ones_row, 1.0)

    # biases: b_qkv[0:1024] per-partition [128, fb]; v-bias folded into the
    # output-projection bias (sum(softmax)=1): bo2 = b_out + bv @ WoutT
    bqk = persist.tile([128, 8], F32, tag="bqk", name="bqk")
    nc.gpsimd.dma_start(out=bqk, in_=b_qkv[0:1024].rearrange("(a b) -> b a", b=128))
    bv_col = persist.tile([128, 4], F32, tag="bv_col", name="bv_col")
    nc.gpsimd.dma_start(
        out=bv_col, in_=b_qkv[1024:1536].rearrange("(a b) -> b a", b=128)
    )
    bvb = persist.tile([128, 4], BF16, tag="bvb", name="bvb")
    nc.vector.tensor_copy(bvb, bv_col)
    bo_f = persist.tile([1, 512], F32, tag="bo_f", name="bo_f")
    nc.gpsimd.dma_start(out=bo_f, in_=b_out.unsqueeze(0))
    bo2 = persist.tile([1, 512], BF16, tag="bo2", name="bo2")
    bo2b = persist.tile([128, 512], F32, tag="bo2b", name="bo2b")
    wu_sb = persist.tile([128, 512], BF16, tag="wu_sb", name="wu_sb")
    nc.vector.memset(wu_sb, 0.0)

    # persistent bf16 operands (single tiles; fewer tags = smaller exit
    # barrier at the kernel tail)
    xT = persist.tile([128, EC, n], BF16, tag="xT", name="xT")
    wqkvT = persist.tile([128, EC, 1536], BF16, tag="wqkvT", name="wqkvT")
    woutT = persist.tile([128, EC, 512], BF16, tag="woutT", name="woutT")
    qkT = persist.tile([128, 8, n], BF16, tag="qkT", name="qkT")
    v_sb = persist.tile([128, NB, 512], BF16, tag="v_sb", name="v_sb")
    outT = persist.tile([128, 4, n], BF16, tag="outT", name="outT")

    # ---------------- phase 0: load (bf16, pre-transposed on host) + QKV ----
    with (
        tc.tile_pool(name="pqkv", bufs=4, space="PSUM") as pqkv_pool,
    ):
        # Critical-path loads first: the very first matmul only needs the
        # j=0 slivers of W-fb0 and x-ncol0, so those are their own tiny DMAs.
        # Then the j-rest of each, the full-width W lines (split across the
        # sync and gpsimd queues so they land before the v matmuls), the
        # remaining x chunks, and W_out last.
        wq_r = w_qkvT.rearrange("(j p) c -> p j c", p=128)
        x_r = xT_d.rearrange("(j p) c -> p j c", p=128)
        # PE warm-up during the DMA dead time: the HAM clock gate opens after
        # ~3.4us of sustained PE activity, so spin a few matmuls on memset
        # data and the first real matmuls run at 2.4GHz instead of 1.2.
        for wi in range(4):
            pw = pqkv_pool.tile([128, 512], F32, tag="qk" if wi % 2 else "v",
                                name="pw")
            nc.tensor.matmul(pw[0:64, :], lhsT=ones_col, rhs=wu_sb,
                             start=True, stop=True)
        # DMA order matches consumption: qk(0) [W fb0 + x], v [W 1024:1536],
        # qk(4..) [W 128:1024], each j=0 sliver first so the opening matmul
        # of each group never waits a full-width line.
        nc.sync.dma_start(out=wqkvT[:, 0:1, 0:128], in_=wq_r[:, 0:1, 0:128])
        nc.scalar.dma_start(out=xT[:, 0:1, 0:512], in_=x_r[:, 0:1, 0:512])
        nc.sync.dma_start(out=wqkvT[:, 1:EC, 0:128], in_=wq_r[:, 1:EC, 0:128])
        nc.scalar.dma_start(out=xT[:, 1:EC, 0:512], in_=x_r[:, 1:EC, 0:512])
        for j in range(EC):
            nc.sync.dma_start(
                out=wqkvT[:, j, 1024:1536], in_=w_qkvT[j * 128:(j + 1) * 128, 1024:1536]
            )
        for j in range(EC):
            nc.sync.dma_start(
                out=wqkvT[:, j, 128:1024], in_=w_qkvT[j * 128:(j + 1) * 128, 128:1024]
            )
        for ncol in range(1, QC):
            nc.scalar.dma_start(
                out=xT[:, :, ncol * 512:(ncol + 1) * 512],
                in_=x_r[:, :, ncol * 512:(ncol + 1) * 512],
            )
        for j in range(EC):
            nc.gpsimd.dma_start(
                out=woutT[:, j, :], in_=w_outT[j * 128:(j + 1) * 128, :]
            )

        def emit_qk_ncol(fb, ncol, pq=None):
            if pq is None:
                pq = pqkv_pool.tile([128, 512], F32, tag="qk", name="pq")
            for j in range(EC):
                nc.tensor.matmul(
                    pq,
                    lhsT=wqkvT[:, j, fb * 128:(fb + 1) * 128],
                    rhs=xT[:, j, ncol * 512:(ncol + 1) * 512],
                    start=(j == 0),
                    stop=(j == EC - 1),
                )
            nc.vector.tensor_scalar_add(
                qkT[:, fb, ncol * 512:(ncol + 1) * 512], pq, bqk[:, fb:fb + 1]
            )

        def emit_qk(fb):
            for ncol in range(QC):
                emit_qk_ncol(fb, ncol)

        def emit_v(nb):
            pv = pqkv_pool.tile([128, 512], F32, tag="v", name="pv")
            for j in range(EC):
                nc.tensor.matmul(
                    pv,
                    lhsT=xT[:, j, nb * 128:(nb + 1) * 128],
                    rhs=wqkvT[:, j, 1024:1536],
                    start=(j == 0),
                    stop=(j == EC - 1),
                )
            nc.vector.tensor_copy(v_sb[:, nb, :], pv)

        emit_qk(0)
        for nb in range(NB):
            emit_v(nb)
        emit_qk(4)
        for fb in (1, 5, 2, 6, 3):
            emit_qk(fb)
        for ncol in (2, 3):
            emit_qk_ncol(7, ncol)

        # bo2 = b_out + bv @ WoutT (one-time; replaces the separate v-bias).
        # Emitted last so its woutT dependency never blocks the qk/v stream.
        pb = pqkv_pool.tile([128, 512], F32, tag="qk", name="pb")
        for j in range(EC):
            nc.tensor.matmul(
                pb[0:1, :], lhsT=bvb[:, j:j + 1], rhs=woutT[:, j, :],
                start=(j == 0), stop=(j == EC - 1),
            )
        nc.vector.tensor_add(bo2, bo_f, pb[0:1, :])
        # broadcast bo2 over 128 partitions once: the finals then add it on
        # the DVE copy instead of spending a PE slot per block on a rank-1
        # ones-lhsT matmul
        pbb = pqkv_pool.tile([128, 512], F32, tag="qk", name="pbb")
        nc.tensor.matmul(pbb, lhsT=ones_row, rhs=bo2, start=True, stop=True)
        nc.vector.tensor_copy(bo2b, pbb)

    # ---------------- phase 1: attention ----------------
    # 2-kb cycles: each cycle computes two kb's score pairs and (one cycle
    # deferred) their av/pd matmuls - 6 pair-slots ~ 1.29us of PE work - and
    # issues exactly one exp per engine (ScalarE 1.11us, DVE 1.22us), so
    # neither engine ever backlogs.  Cycle 0 of each qc is [ACT,ACT]: the
    # DVE gap there absorbs the previous qc's recip+mul, so the po/pd WAR
    # at the qc boundary resolves before av(kb0) needs the banks.  Score
    # streams rotate by GLOBAL kb (gkb%3): the stream a new qc's kb0 reuses
    # was exp'd 3 kb earlier, not at the previous qc's end.
    STAG = ("sA", "sB", "sC")
    cycles = [tuple(range(s, s + 2)) for s in range(0, KB, 2)]
    with (
        tc.tile_pool(name="ps", bufs=1, space="PSUM") as s_pool,
        tc.tile_pool(name="po", bufs=1, space="PSUM") as o_pool,
        tc.tile_pool(name="se", bufs=3) as e_pool,
        tc.tile_pool(name="sr", bufs=2) as r_pool,
        tc.tile_pool(name="sy", bufs=4) as y_pool,
    ):
        def emit_final(nb, ftag):
            pf = o_pool.tile([128, 512], F32, tag=ftag, name="pf")
            for pp in range(4):
                nc.tensor.matmul(
                    pf, lhsT=outT[:, pp, nb * 128:(nb + 1) * 128],
                    rhs=woutT[:, pp, :], start=(pp == 0), stop=(pp == 3),
                )
            ys = y_pool.tile([128, 512], F32, tag="y", name="ys")
            nc.vector.tensor_add(ys, pf, bo2b)
            nc.sync.dma_start(out=y[nb * 128:(nb + 1) * 128, :], in_=ys)

        # deferred-work queue: (weight in PE pair-slots, closure).  Each
        # cycle flushes ~one cycle's worth so avs trail their exps by one
        # cycle and the finals spread instead of bunching.
        work = []

        def flush(budget=5):
            spent = 0
            while work and spent < budget:
                wt, w = work.pop(0)
                w()
                spent += wt

        # all of qk(7) fills the first qc's pipeline-fill bubble (the PE has
        # no deferred avs in its first cycles, and an idle gap there lets the
        # HAM clock gate re-throttle); its PSUM rides the o/d banks, whose
        # first real write av(kb0) is emitted after these flush
        def qk7_fill(ncol, ftag):
            pq = o_pool.tile([128, 512], F32, tag=ftag, name="pq7")
            emit_qk_ncol(7, ncol, pq=pq)

        for ncol in range(2):
            work.append((4, lambda ncol=ncol, t=("o" if ncol % 2 == 0 else "d"):
                         qk7_fill(ncol, t)))

        for p in range(4):
            for qc in range(QC):
                qs = slice(qc * 512, (qc + 1) * 512)
                po = o_pool.tile([128, 512], F32, tag="o", name="po")
                pd = o_pool.tile([128, 512], F32, tag="d", name="pd")

                def po_one(e, kb, po=po, p=p):
                    first, last = (kb == 0), (kb == KB - 1)
                    nc.tensor.matmul(
                        po[0:64, :], lhsT=v_sb[:, kb, p * 128:p * 128 + 64],
                        rhs=e[:, 0, :], start=first, stop=last,
                        skip_group_check=True,
                    )
                    nc.tensor.matmul(
                        po[64:128, :],
                        lhsT=v_sb[:, kb, p * 128 + 64:(p + 1) * 128],
                        rhs=e[:, 1, :], start=first, stop=last,
                        skip_group_check=True,
                    )

                def pd_one(e, kb, pd=pd):
                    first, last = (kb == 0), (kb == KB - 1)
                    nc.tensor.matmul(
                        pd[0:64, :], lhsT=ones_col, rhs=e[:, 0, :],
                        start=first, stop=last, skip_group_check=True,
                    )
                    nc.tensor.matmul(
                        pd[64:128, :], lhsT=ones_col, rhs=e[:, 1, :],
                        start=first, stop=last, skip_group_check=True,
                    )

                def av2(t_i, t_j, po_one=po_one, pd_one=pd_one):
                    # grouped po,po,pd,pd: the two pd pairs load IDENTICAL
                    # ones weights back-to-back (dedupe/prefetch friendly).
                    # On the last k-block run the pds first so the epilogue
                    # reciprocal starts two slots earlier.
                    (e_i, kb_i), (e_j, kb_j) = t_i, t_j
                    if kb_j == KB - 1:
                        pd_one(e_i, kb_i); pd_one(e_j, kb_j)
                        po_one(e_i, kb_i); po_one(e_j, kb_j)
                    else:
                        po_one(e_i, kb_i); po_one(e_j, kb_j)
                        pd_one(e_i, kb_i); pd_one(e_j, kb_j)

                def normalize(po=po, pd=pd, p=p, qs=qs):
                    rc = r_pool.tile([128, 512], F32, tag="rc", name="rc")
                    nc.vector.reciprocal_approx_fast(rc, pd)
                    nc.vector.tensor_mul(outT[:, p, qs], po, rc)

                for ci, cyc in enumerate(cycles):
                    tiles = []
                    for i, kb in enumerate(cyc):
                        gkb = ((p * QC + qc) * KB + kb)
                        st = STAG[gkb % 3]
                        ks = slice(kb * 128, (kb + 1) * 128)
                        S = s_pool.tile([128, 2, 512], F32, tag=st, name="S")
                        nc.tensor.matmul(
                            S[:, 0, :], lhsT=qkT[0:64, 4 + p, ks],
                            rhs=qkT[0:64, p, qs], start=True, stop=True,
                        )
                        nc.tensor.matmul(
                            S[:, 1, :], lhsT=qkT[64:128, 4 + p, ks],
                            rhs=qkT[64:128, p, qs], start=True, stop=True,
                        )
                        e = e_pool.tile([128, 2, 512], BF16, tag="e" + st, name="e")
                        if ci == 0:
                            on_dve = False          # [A, A] boundary cycle
                        elif ci % 2 == 1:
                            on_dve = (i == 0)       # [D, A]
                        else:
                            on_dve = (i == 1)       # [A, D]
                        if on_dve:
                            nc.vector._custom_dve(
                                EXP16, out=e, in0=S, s0=EC2, s1=EC1, imm2=EC0
                            )
                        else:
                            nc.scalar.activation(
                                e, S, mybir.ActivationFunctionType.Exp, scale=0.125,
                            )
                        tiles.append((e, kb))
                    flush()
                    work.append(
                        (4, lambda a=tiles[0], b=tiles[1], f=av2: f(a, b))
                    )
                # normalization and (on the last pair) the finished output
                # rows join the deferred queue so the next qc's scores/exps
                # stay ahead of them
                work.append((0, normalize))
                if p == 3:
                    for i, nb in enumerate(range(qc * 4, qc * 4 + 4)):
                        work.append(
                            (4, lambda nb=nb, t=("o" if i % 2 == 0 else "d"),
                                emit_final=emit_final: emit_final(nb, t))
                        )
        while work:
            flush()
    persist_cm.__exit__(None, None, None)


def build(n=N_SEQ):
    nc = bacc.Bacc("TRN2", target_bir_lowering=False, debug=False)
    xT_d = nc.dram_tensor("xT", [E, n], BF16, kind="ExternalInput").ap()
    w_qkvT = nc.dram_tensor("w_qkvT", [E, 3 * E], BF16, kind="ExternalInput").ap()
    b_qkv = nc.dram_tensor("b_qkv", [3 * E], F32, kind="ExternalInput").ap()
    w_outT = nc.dram_tensor("w_outT", [E, E], BF16, kind="ExternalInput").ap()
    b_out = nc.dram_tensor("b_out", [E], F32, kind="ExternalInput").ap()
    y = nc.dram_tensor("y", [n, E], F32, kind="ExternalOutput").ap()
    with tile.TileContext(nc) as tc:
        _emit(tc, nc, xT_d, w_qkvT, b_qkv, w_outT, b_out, y, n)
    nc.compile()
    return nc


_NC_CACHE = {}


def _get_nc(n):
    if n not in _NC_CACHE:
        _NC_CACHE[n] = build(n)
    return _NC_CACHE[n]


def _in_maps(seq, W_qkv, b_qkv, W_out, b_out):
    import ml_dtypes

    bf16 = ml_dtypes.bfloat16
    seq = np.asarray(seq, np.float32)
    wqT = np.ascontiguousarray(np.asarray(W_qkv, np.float32).T.astype(bf16))
    bq = np.ascontiguousarray(np.asarray(b_qkv, np.float32))
    woT = np.ascontiguousarray(np.asarray(W_out, np.float32).T.astype(bf16))
    bo = np.ascontiguousarray(np.asarray(b_out, np.float32))
    return [
        {
            "xT": np.ascontiguousarray(seq[:, b, :].T.astype(bf16)),  # [E, n]
            "w_qkvT": wqT,
            "b_qkv": bq,
            "w_outT": woT,
            "b_out": bo,
        }
        for b in range(seq.shape[1])
    ]


def run(seq, W_qkv, b_qkv, W_out, b_out, trace=False):
    """Returns (out [n, bs, e] fp32, BassKernelResults)."""
    from concourse.bass_utils import run_bass_kernel_spmd

    seq = np.asarray(seq, np.float32)
    n, bs, e = seq.shape
    nc = _get_nc(n)
    res = run_bass_kernel_spmd(
        nc,
        _in_maps(seq, W_qkv, b_qkv, W_out, b_out),
        core_ids=list(range(N_CORES)),
        trace=trace,
    )
    out = np.empty((n, bs, e), np.float32)
    for b in range(bs):
        out[:, b, :] = res.results[b]["y"]
    return out, res


def kernel(seq, W_qkv, b_qkv, W_out, b_out):
    out, _ = run(seq, W_qkv, b_qkv, W_out, b_out)
    return out


# revision 41
# speedup vs baseline: 1.0147x; 1.0024x over previous
"""Multi-head self-attention Trainium2 kernel (Bass/Tile), batch-sharded SPMD.

Problem: seq [2048, 8, 512] fp32, fused QKV (W_qkv [1536,512], b_qkv [1536]),
H=8 heads of HD=64, full softmax attention, out proj (W_out [512,512], b_out).

Sharding: batch (bs=8) across 8 NeuronCores, one batch element per core, no
collectives. The host pre-transposes per-core x -> xT [e, n] and the weights
(and casts them to bf16), scatters, and gathers y -> [n, bs, e].

Per-core dataflow (n=2048, E=512, all matmuls bf16 with fp32 PSUM):
  qkT [f, n] <- WqkvT.T @ xT   (f in [0,1024): q|k features; each 128-row
                tile holds a head PAIR: rows 0:64 head 2p, 64:128 head 2p+1)
  v   [n, f] <- xT.T @ WvT     (no bias matmul: since sum(softmax)=1, the
                v-bias is folded into the out-proj bias bo2 = b_out+bv@WoutT)
  attention, per head pair p, per q-chunk (512 cols), 3-kb cycles:
    scoresT[k,q]: row-paired matmuls into per-kb streams sA/sB/sC
    exp: balanced across ScalarE (exact ACTIVATE) and DVE (custom EXP16_ANT,
         exp(s/8) ~ poly^16) at 9:7 per qc - the v1 split (ACT ~2.2us/cycle
         vs the 1.94us PE cycle) made ACT the wall and stalled each cycle
    av/denominator (deferred one cycle): col-paired matmuls po += v.T @ e,
        pd += ones.T @ e; epilogue rc=1/pd on DVE, outT = po*rc on DVE
  y [n, f] = outT.T @ WoutT + b: bias enters as a K=1 ones_row x bo2 matmul
    into PSUM, so the epilogue is a pure PSUM->SBUF copy split ScalarE/DVE.

Changes vs the 302.7us prior version (trace-driven; measured 297.0us,
rel err 7.0e-3):
  - 2-kb cycles (was 3): each cycle carries exactly one ScalarE exp and one
    DVE exp (1114/1224ns, both under the ~1.3us PE cycle), killing the
    per-cycle integer imbalance (2 ACT exps = 2.23us/cycle vs a 1.94us
    3-kb PE cycle) that stalled the PE ~700-900ns every cycle
  - cycle 0 of each qc is [ACT,ACT]: the DVE gap absorbs the previous qc's
    recip+mul so the po/pd WAR at the qc boundary stops stalling the PE
  - score streams rotate by GLOBAL kb index (gkb%3) so the first stream a
    new qc reuses was exp'd 3 kb earlier, not at the previous qc's end
  - exp split 9:7 ScalarE:DVE per qc (engine-balanced incl. normalize)
  - startup: j=0 slivers of W-fb0/x-ncol0 land first; W ordered fb0 -> v
    slice -> the rest to match qk(0) -> v -> qk(4..) consumption; PE warm-up
    matmuls during the DMA dead time so the HAM clock gate opens early
  - qk(7) n0/n1 ride the attention deferred-work queue (their PSUM borrows
    the o/d banks) to fill the first qc's exp pipeline-fill bubble
  - av work grouped po,po,pd,pd per 2-kb cycle (fewer worst-case LDWEIGHTS
    quadrant transitions; walrus does NOT dedupe the identical ones-loads)
  - persistent tiles consolidated (qkT/v/outT single tiles): smaller exit
    barrier at the tail

Where the remaining time goes (per the NTFF profile): PE streaming floor
~218us + ~30us exposed LDWEIGHTS at paired-matmul quadrant transitions
(tile_position'd loads cannot use the background weight buffer) + ~47us
projection phase + ~6us fixed runtime setup + ~6us tail teardown.  The
attention phase runs cycle-lockstep (3 single-buffered score streams), so
per-cycle time = max(PE, exp engine) every cycle; PE and the two exp
engines are co-bound within ~10%.  Rejected with numbers: fp8/DoubleRow
anywhere in the value path (~2.5% rel err - quantization of a random
weighted sum passes through 1:1, gate is 2e-2), GpSimd softmax-denominator
offload (partition_all_reduce 13.2us per [128,2048]; tensor_add 2.1us per
[128,1024] - capacity-dead), and e-pair pre-summing for pd (PE savings in
light cycles don't transfer across the lockstep; measured neutral).
"""

import numpy as np

import concourse.bass as bass
import concourse.mybir as mybir
import concourse.tile as tile
from concourse import bacc
from concourse import dve_ops
from concourse.dve_spec import Spec, Src0, C0, C1, C2, sq
from concourse.dve_uop import DveOpSpec
from concourse.dve_ops import DveOp
from concourse.dve_spec import lower as dve_lower

F32 = mybir.dt.float32
BF16 = mybir.dt.bfloat16

N_SEQ, BS, E, H, HD = 2048, 8, 512, 8, 64
N_CORES = 8

# exp(0.125*s) ~ ((EC2*s + EC1)*s + EC0)^16, minimax-fitted on s in [-40, 40]
# (observed raw-score range is [-36.3, 37.2]); max rel err 2.05e-2 which lands
# at ~6.7e-3 end-to-end with 7/16 of k-blocks routed to the DVE.
EC2, EC1, EC0 = 3.03313468e-05, 7.90702397e-03, 1.00029378e+00


def _register_exp16():
    """Register the custom DVE op (documented extension point in dve_ops)."""
    if "EXP16_ANT" in dve_ops._SUB_OPCODE_FOR_NAME:
        return next(o for o in dve_ops.OPS if o.name == "EXP16_ANT")
    body = sq(sq(sq(sq((Src0 * C0 + C1) * Src0 + C2))))

    def ref(in0, in1, s0, s1, imm2):
        p = (in0.astype(np.float32) * s0 + s1) * in0 + imm2
        for _ in range(4):
            p = p * p
        return p

    spec = Spec(body=body, reference=ref)
    shas = {}
    for ver in ("v3", "v4"):
        uops = dve_lower(spec, ver=ver)
        shas[ver] = DveOpSpec(name="EXP16_ANT", opcode=0, uops=uops, rd1_en=False).sha(ver)
    op = DveOp("EXP16_ANT", spec, subdim=False, uops_sha=shas)
    dve_ops.OPS.append(op)
    dve_ops.CUSTOM_DVE_SPECS[op.name] = spec
    dve_ops._SUB_OPCODE_FOR_NAME[op.name] = (
        dve_ops._CUSTOM_DVE_ROW_BASE + len(dve_ops.OPS) - 1
    )
    return op


EXP16 = _register_exp16()


def _emit(tc, nc, xT_d, w_qkvT, b_qkv, w_outT, b_out, y, n):
    NB = n // 128   # token blocks
    QC = n // 512   # q chunks
    KB = n // 128   # k blocks
    EC = E // 128   # e chunks

    persist_cm = tc.tile_pool(name="persist", bufs=1)
    persist = persist_cm.__enter__()

    ones_col = persist.tile([128, 64], BF16, tag="ones_col", name="ones_col")
    nc.vector.memset(ones_col, 1.0)
    ones_row = persist.tile([1, 128], BF16, tag="ones_row", name="ones_row")
    nc.vector.memset(ones_row, 1.0)

    # biases: b_qkv[0:1024] per-partition [128, fb]; v-bias folded into the
    # output-projection bias (sum(softmax)=1): bo2 = b_out + bv @ WoutT
    bqk = persist.tile([128, 8], F32, tag="bqk", name="bqk")
    nc.gpsimd.dma_start(out=bqk, in_=b_qkv[0:1024].rearrange("(a b) -> b a", b=128))
    bv_col = persist.tile([128, 4], F32, tag="bv_col", name="bv_col")
    nc.gpsimd.dma_start(
        out=bv_col, in_=b_qkv[1024:1536].rearrange("(a b) -> b a", b=128)
    )
    bvb = persist.tile([128, 4], BF16, tag="bvb", name="bvb")
    nc.vector.tensor_copy(bvb, bv_col)
    bo_f = persist.tile([1, 512], F32, tag="bo_f", name="bo_f")
    nc.gpsimd.dma_start(out=bo_f, in_=b_out.unsqueeze(0))
    bo2 = persist.tile([1, 512], BF16, tag="bo2", name="bo2")
    bo2b = persist.tile([128, 512], F32, tag="bo2b", name="bo2b")
    wu_sb = persist.tile([128, 512], BF16, tag="wu_sb", name="wu_sb")
    nc.vector.memset(wu_sb, 0.0)

    # persistent bf16 operands (single tiles; fewer tags = smaller exit
    # barrier at the kernel tail)
    xT = persist.tile([128, EC, n], BF16, tag="xT", name="xT")
    wqkvT = persist.tile([128, EC, 1536], BF16, tag="wqkvT", name="wqkvT")
    woutT = persist.tile([128, EC, 512], BF16, tag="woutT", name="woutT")
    qkT = persist.tile([128, 8, n], BF16, tag="qkT", name="qkT")
    v_sb = persist.tile([128, NB, 512], BF16, tag="v_sb", name="v_sb")
    outT = persist.tile([128, 4, n], BF16, tag="outT", name="outT")

    # ---------------- phase 0: load (bf16, pre-transposed on host) + QKV ----
    with (
        tc.tile_pool(name="pqkv", bufs=4, space="PSUM") as pqkv_pool,
    ):
        # Critical-path loads first: the very first matmul only needs the
        # j=0 slivers of W-fb0 and x-ncol0, so those are their own tiny DMAs.
        # Then the j-rest of each, the full-width W lines (split across the
        # sync and gpsimd queues so they land before the v matmuls), the
        # remaining x chunks, and W_out last.
        wq_r = w_qkvT.rearrange("(j p) c -> p j c", p=128)
        x_r = xT_d.rearrange("(j p) c -> p j c", p=128)
        # PE warm-up during the DMA dead time: the HAM clock gate opens after
        # ~3.4us of sustained PE activity, so spin a few matmuls on memset
        # data and the first real matmuls run at 2.4GHz instead of 1.2.
        for wi in range(4):
            pw = pqkv_pool.tile([128, 512], F32, tag="qk" if wi % 2 else "v",
                                name="pw")
            nc.tensor.matmul(pw[0:64, :], lhsT=ones_col, rhs=wu_sb,
                             start=True, stop=True)
        # DMA order matches consumption: qk(0) [W fb0 + x], v [W 1024:1536],
        # qk(4..) [W 128:1024], each j=0 sliver first so the opening matmul
        # of each group never waits a full-width line.
        nc.sync.dma_start(out=wqkvT[:, 0:1, 0:128], in_=wq_r[:, 0:1, 0:128])
        nc.scalar.dma_start(out=xT[:, 0:1, 0:512], in_=x_r[:, 0:1, 0:512])
        nc.sync.dma_start(out=wqkvT[:, 1:EC, 0:128], in_=wq_r[:, 1:EC, 0:128])
        nc.scalar.dma_start(out=xT[:, 1:EC, 0:512], in_=x_r[:, 1:EC, 0:512])
        for j in range(EC):
            nc.sync.dma_start(
                out=wqkvT[:, j, 1024:1536], in_=w_qkvT[j * 128:(j + 1) * 128, 1024:1536]
            )
        for j in range(EC):
            nc.sync.dma_start(
                out=wqkvT[:, j, 128:1024], in_=w_qkvT[j * 128:(j + 1) * 128, 128:1024]
            )
        for ncol in range(1, QC):
            nc.scalar.dma_start(
                out=xT[:, :, ncol * 512:(ncol + 1) * 512],
                in_=x_r[:, :, ncol * 512:(ncol + 1) * 512],
            )
        for j in range(EC):
            nc.gpsimd.dma_start(
                out=woutT[:, j, :], in_=w_outT[j * 128:(j + 1) * 128, :]
            )

        def emit_qk_ncol(fb, ncol, pq=None):
            if pq is None:
                pq = pqkv_pool.tile([128, 512], F32, tag="qk", name="pq")
            for j in range(EC):
                nc.tensor.matmul(
                    pq,
                    lhsT=wqkvT[:, j, fb * 128:(fb + 1) * 128],
                    rhs=xT[:, j, ncol * 512:(ncol + 1) * 512],
                    start=(j == 0),
                    stop=(j == EC - 1),
                )
            nc.vector.tensor_scalar_add(
                qkT[:, fb, ncol * 512:(ncol + 1) * 512], pq, bqk[:, fb:fb + 1]
            )

        def emit_qk(fb):
            for ncol in range(QC):
                emit_qk_ncol(fb, ncol)

        def emit_v(nb):
            pv = pqkv_pool.tile([128, 512], F32, tag="v", name="pv")
            for j in range(EC):
                nc.tensor.matmul(
                    pv,
                    lhsT=xT[:, j, nb * 128:(nb + 1) * 128],
                    rhs=wqkvT[:, j, 1024:1536],
                    start=(j == 0),
                    stop=(j == EC - 1),
                )
            nc.vector.tensor_copy(v_sb[:, nb, :], pv)

        emit_qk(0)
        for nb in range(NB):
            emit_v(nb)
        emit_qk(4)
        for fb in (1, 5, 2, 6, 3):
            emit_qk(fb)
        for ncol in (2, 3):
            emit_qk_ncol(7, ncol)

        # bo2 = b_out + bv @ WoutT (one-time; replaces the separate v-bias).
        # Emitted last so its woutT dependency never blocks the qk/v stream.
        pb = pqkv_pool.tile([128, 512], F32, tag="qk", name="pb")
        for j in range(EC):
            nc.tensor.matmul(
                pb[0:1, :], lhsT=bvb[:, j:j + 1], rhs=woutT[:, j, :],
                start=(j == 0), stop=(j == EC - 1),
            )
        nc.vector.tensor_add(bo2, bo_f, pb[0:1, :])
        # broadcast bo2 over 128 partitions once: the finals then add it on
        # the DVE copy instead of spending a PE slot per block on a rank-1
        # ones-lhsT matmul
        pbb = pqkv_pool.tile([128, 512], F32, tag="qk", name="pbb")
        nc.tensor.matmul(pbb, lhsT=ones_row, rhs=bo2, start=True, stop=True)
        nc.vector.tensor_copy(bo2b, pbb)

    # ---------------- phase 1: attention ----------------
    # 2-kb cycles: each cycle computes two kb's score pairs and (one cycle
    # deferred) their av/pd matmuls - 6 pair-slots ~ 1.29us of PE work - and
    # issues exactly one exp per engine (ScalarE 1.11us, DVE 1.22us), so
    # neither engine ever backlogs.  Cycle 0 of each qc is [ACT,ACT]: the
    # DVE gap there absorbs the previous qc's recip+mul, so the po/pd WAR
    # at the qc boundary resolves before av(kb0) needs the banks.  Score
    # streams rotate by GLOBAL kb (gkb%3): the stream a new qc's kb0 reuses
    # was exp'd 3 kb earlier, not at the previous qc's end.
    STAG = ("sA", "sB", "sC")
    cycles = [tuple(range(s, s + 2)) for s in range(0, KB, 2)]
    with (
        tc.tile_pool(name="ps", bufs=1, space="PSUM") as s_pool,
        tc.tile_pool(name="po", bufs=1, space="PSUM") as o_pool,
        tc.tile_pool(name="se", bufs=3) as e_pool,
        tc.tile_pool(name="sr", bufs=2) as r_pool,
        tc.tile_pool(name="sy", bufs=4) as y_pool,
    ):
        def emit_final(nb, ftag):
            pf = o_pool.tile([128, 512], F32, tag=ftag, name="pf")
            for pp in range(4):
                nc.tensor.matmul(
                    pf, lhsT=outT[:, pp, nb * 128:(nb + 1) * 128],
                    rhs=woutT[:, pp, :], start=(pp == 0), stop=(pp == 3),
                )
            ys = y_pool.tile([128, 512], F32, tag="y", name="ys")
            nc.vector.tensor_add(ys, pf, bo2b)
            nc.sync.dma_start(out=y[nb * 128:(nb + 1) * 128, :], in_=ys)

        # deferred-work queue: (weight in PE pair-slots, closure).  Each
        # cycle flushes ~one cycle's worth so avs trail their exps by one
        # cycle and the finals spread instead of bunching.
        work = []

        def flush(budget=5):
            spent = 0
            while work and spent < budget:
                wt, w = work.pop(0)
                w()
                spent += wt

        # all of qk(7) fills the first qc's pipeline-fill bubble (the PE has
        # no deferred avs in its first cycles, and an idle gap there lets the
        # HAM clock gate re-throttle); its PSUM rides the o/d banks, whose
        # first real write av(kb0) is emitted after these flush
        def qk7_fill(ncol, ftag):
            pq = o_pool.tile([128, 512], F32, tag=ftag, name="pq7")
            emit_qk_ncol(7, ncol, pq=pq)

        for ncol in range(2):
            work.append((4, lambda ncol=ncol, t=("o" if ncol % 2 == 0 else "d"):
                         qk7_fill(ncol, t)))

        for p in range(4):
            for qc in range(QC):
                qs = slice(qc * 512, (qc + 1) * 512)
                po = o_pool.tile([128, 512], F32, tag="o", name="po")
                pd = o_pool.tile([128, 512], F32, tag="d", name="pd")

                def po_one(e, kb, po=po, p=p):
                    first, last = (kb == 0), (kb == KB - 1)
                    nc.tensor.matmul(
                        po[0:64, :], lhsT=v_sb[:, kb, p * 128:p * 128 + 64],
                        rhs=e[:, 0, :], start=first, stop=last,
                        skip_group_check=True,
                    )
                    nc.tensor.matmul(
                        po[64:128, :],
                        lhsT=v_sb[:, kb, p * 128 + 64:(p + 1) * 128],
                        rhs=e[:, 1, :], start=first, stop=last,
                        skip_group_check=True,
                    )

                def pd_one(e, kb, pd=pd):
                    first, last = (kb == 0), (kb == KB - 1)
                    nc.tensor.matmul(
                        pd[0:64, :], lhsT=ones_col, rhs=e[:, 0, :],
                        start=first, stop=last, skip_group_check=True,
                    )
                    nc.tensor.matmul(
                        pd[64:128, :], lhsT=ones_col, rhs=e[:, 1, :],
                        start=first, stop=last, skip_group_check=True,
                    )

                def av2(t_i, t_j, po_one=po_one, pd_one=pd_one):
                    # grouped po,po,pd,pd in steady state (fewer worst-case
                    # LDWEIGHTS quadrant transitions).  On the last k-block,
                    # pds first so the epilogue reciprocal starts two slots
                    # earlier.  On the FIRST k-block (qc boundary), order by
                    # dependency readiness: pd0 waits only the previous qc's
                    # recip (~1.4us) and exp0, po0 waits its mul (~2.1us),
                    # pd1 waits the late second boundary exp (~2.6us).
                    (e_i, kb_i), (e_j, kb_j) = t_i, t_j
                    if kb_i == 0:
                        pd_one(e_i, kb_i); po_one(e_i, kb_i)
                        pd_one(e_j, kb_j); po_one(e_j, kb_j)
                    elif kb_j == KB - 1:
                        pd_one(e_i, kb_i); pd_one(e_j, kb_j)
                        po_one(e_i, kb_i); po_one(e_j, kb_j)
                    else:
                        po_one(e_i, kb_i); po_one(e_j, kb_j)
                        pd_one(e_i, kb_i); pd_one(e_j, kb_j)

                def normalize(po=po, pd=pd, p=p, qs=qs):
                    rc = r_pool.tile([128, 512], F32, tag="rc", name="rc")
                    nc.vector.reciprocal_approx_fast(rc, pd)
                    nc.vector.tensor_mul(outT[:, p, qs], po, rc)

                for ci, cyc in enumerate(cycles):
                    tiles = []
                    for i, kb in enumerate(cyc):
                        gkb = ((p * QC + qc) * KB + kb)
                        st = STAG[gkb % 3]
                        ks = slice(kb * 128, (kb + 1) * 128)
                        S = s_pool.tile([128, 2, 512], F32, tag=st, name="S")
                        nc.tensor.matmul(
                            S[:, 0, :], lhsT=qkT[0:64, 4 + p, ks],
                            rhs=qkT[0:64, p, qs], start=True, stop=True,
                        )
                        nc.tensor.matmul(
                            S[:, 1, :], lhsT=qkT[64:128, 4 + p, ks],
                            rhs=qkT[64:128, p, qs], start=True, stop=True,
                        )
                        e = e_pool.tile([128, 2, 512], BF16, tag="e" + st, name="e")
                        if ci == 0:
                            on_dve = False          # [A, A] boundary cycle
                        elif ci % 2 == 1:
                            on_dve = (i == 0)       # [D, A]
                        else:
                            on_dve = (i == 1)       # [A, D]
                        if on_dve:
                            nc.vector._custom_dve(
                                EXP16, out=e, in0=S, s0=EC2, s1=EC1, imm2=EC0
                            )
                        else:
                            nc.scalar.activation(
                                e, S, mybir.ActivationFunctionType.Exp, scale=0.125,
                            )
                        tiles.append((e, kb))
                    flush()
                    work.append(
                        (4, lambda a=tiles[0], b=tiles[1], f=av2: f(a, b))
                    )
                # normalization and (on the last pair) the finished output
                # rows join the deferred queue so the next qc's scores/exps
                # stay ahead of them
                work.append((0, normalize))
                if p == 3:
                    for i, nb in enumerate(range(qc * 4, qc * 4 + 4)):
                        work.append(
                            (4, lambda nb=nb, t=("o" if i % 2 == 0 else "d"),
                                emit_final=emit_final: emit_final(nb, t))
                        )
        while work:
            flush()
    persist_cm.__exit__(None, None, None)


def build(n=N_SEQ):
    nc = bacc.Bacc("TRN2", target_bir_lowering=False, debug=False)
    xT_d = nc.dram_tensor("xT", [E, n], BF16, kind="ExternalInput").ap()
    w_qkvT = nc.dram_tensor("w_qkvT", [E, 3 * E], BF16, kind="ExternalInput").ap()
    b_qkv = nc.dram_tensor("b_qkv", [3 * E], F32, kind="ExternalInput").ap()
    w_outT = nc.dram_tensor("w_outT", [E, E], BF16, kind="ExternalInput").ap()
    b_out = nc.dram_tensor("b_out", [E], F32, kind="ExternalInput").ap()
    y = nc.dram_tensor("y", [n, E], F32, kind="ExternalOutput").ap()
    with tile.TileContext(nc) as tc:
        _emit(tc, nc, xT_d, w_qkvT, b_qkv, w_outT, b_out, y, n)
    nc.compile()
    return nc


_NC_CACHE = {}


def _get_nc(n):
    if n not in _NC_CACHE:
        _NC_CACHE[n] = build(n)
    return _NC_CACHE[n]


def _in_maps(seq, W_qkv, b_qkv, W_out, b_out):
    import ml_dtypes

    bf16 = ml_dtypes.bfloat16
    seq = np.asarray(seq, np.float32)
    wqT = np.ascontiguousarray(np.asarray(W_qkv, np.float32).T.astype(bf16))
    bq = np.ascontiguousarray(np.asarray(b_qkv, np.float32))
    woT = np.ascontiguousarray(np.asarray(W_out, np.float32).T.astype(bf16))
    bo = np.ascontiguousarray(np.asarray(b_out, np.float32))
    return [
        {
            "xT": np.ascontiguousarray(seq[:, b, :].T.astype(bf16)),  # [E, n]
            "w_qkvT": wqT,
            "b_qkv": bq,
            "w_outT": woT,
            "b_out": bo,
        }
        for b in range(seq.shape[1])
    ]


def run(seq, W_qkv, b_qkv, W_out, b_out, trace=False):
    """Returns (out [n, bs, e] fp32, BassKernelResults)."""
    from concourse.bass_utils import run_bass_kernel_spmd

    seq = np.asarray(seq, np.float32)
    n, bs, e = seq.shape
    nc = _get_nc(n)
    res = run_bass_kernel_spmd(
        nc,
        _in_maps(seq, W_qkv, b_qkv, W_out, b_out),
        core_ids=list(range(N_CORES)),
        trace=trace,
    )
    out = np.empty((n, bs, e), np.float32)
    for b in range(bs):
        out[:, b, :] = res.results[b]["y"]
    return out, res


def kernel(seq, W_qkv, b_qkv, W_out, b_out):
    out, _ = run(seq, W_qkv, b_qkv, W_out, b_out)
    return out
